# revision 19
# baseline (speedup 1.0000x reference)
"""DyHGCN_H forward as a Trainium2 Bass kernel, SPMD over 8 NeuronCores.

Sharding:
  - GCN + transformer: data-parallel over batch (2 sequences per core). The
    GCN is evaluated sparsely: only the (time-slice, node) pairs actually
    gathered by dyemb are computed, via their 2-hop in-neighborhoods.
  - Aggregation is a gather + one-hot matmul segment-sum (edges sorted by
    destination, packed in 2048-edge bins, accumulated in PSUM).
  - x2 is all-gathered (72 x 3200), then the [30000, 72] output projection +
    previous-user mask run tensor-parallel over the vocab axis (3750/core).
"""
import os
import sys
from contextlib import ExitStack

for _p in ("/opt/trn_rl_repo", "/root/.axon_site/_ro/trn_rl_repo"):
    if os.path.isdir(_p) and _p not in sys.path:
        sys.path.insert(0, _p)

import numpy as np
from concourse import bacc, mybir, tile
from concourse import bass_utils
from concourse.masks import make_identity

f32 = mybir.dt.float32
i16 = mybir.dt.int16
i32 = mybir.dt.int32
i8 = mybir.dt.int8

B, L1TOT, N, D, T = 16, 201, 30000, 64, 8
L, INP, H, DK = 200, 72, 8, 64
STEP = 5
NCORES, NBC = 8, 2          # cores, batches per core
NSH = N // NCORES           # vocab shard: 3750
NEG = float(-2 ** 32 + 1)
BIN = 2048                  # edges per superblock (one 128-wide PSUM window)
NBLK = BIN // 128           # matmul blocks per bin
CHUNK = 8192                # edges per dma_gather call
QT_ROWS = (128, 72)         # rows per q-tile (200 = 128 + 72)
VT = [(v, min(512, NSH - v)) for v in range(0, NSH, 512)]  # vocab tiles

_CACHE = {}


# ----------------------------------------------------------------------
# host-side index packing
# ----------------------------------------------------------------------

def _wrap16(idx, cap):
    """idx j -> [16, cap/16] at (j%16, j//16), tiled to 128 partitions."""
    buf = np.full(cap, -1, np.int16)
    buf[: len(idx)] = np.asarray(idx, np.int16)
    return np.tile(buf.reshape(cap // 16, 16).T, (8, 1))


def _wrap128(vals, cap, fill):
    v = np.full(cap, fill, np.float32)
    v[: len(vals)] = vals
    return np.ascontiguousarray(v.reshape(cap // 128, 128).T)


class _Packer:
    """Pack per-destination edge groups into BIN-edge superblocks, each
    covering <=128 distinct destinations; assigns padded local ids."""

    def __init__(self):
        self.src = []
        self.dl = []
        self.cf = []
        self.norm_d = []       # per padded id (len nbins*128)
        self.nbins = 0
        self._used_e = 0
        self._used_d = 0

    def _flush(self):
        if self._used_e or self._used_d:
            pad = BIN - self._used_e
            if pad:
                self.src.append(np.zeros(pad, np.int64))
                self.dl.append(np.full(pad, -1.0, np.float32))
                self.cf.append(np.zeros(pad, np.float32))
            self.norm_d.extend([0.0] * (128 - self._used_d))
            self.nbins += 1
            self._used_e = 0
            self._used_d = 0

    def add(self, srcs, cfs, norm_dst):
        k = len(srcs)
        assert 0 < k <= BIN, k
        if self._used_e + k > BIN or self._used_d >= 128:
            self._flush()
        pid = self.nbins * 128 + self._used_d
        self.src.append(np.asarray(srcs, np.int64))
        self.cf.append(np.asarray(cfs, np.float32))
        self.dl.append(np.full(k, float(self._used_d), np.float32))
        self.norm_d.append(float(norm_dst))
        self._used_e += k
        self._used_d += 1
        return pid

    def finish(self, nbins_cap):
        self._flush()
        assert self.nbins <= nbins_cap, (self.nbins, nbins_cap)
        e_cap = nbins_cap * BIN
        src = np.concatenate(self.src) if self.src else np.zeros(0, np.int64)
        dl = np.concatenate(self.dl) if self.dl else np.zeros(0, np.float32)
        cf = np.concatenate(self.cf) if self.cf else np.zeros(0, np.float32)
        nd = np.zeros(nbins_cap * 128, np.float32)
        nd[: len(self.norm_d)] = self.norm_d
        src_full = np.zeros(e_cap, np.int64)   # pad with valid idx 0
        src_full[: len(src)] = src
        return (_wrap16(src_full, e_cap), _wrap128(dl, e_cap, -1.0),
                _wrap128(cf, e_cap, 0.0), nd)


def _host_prep(inputs):
    inp = np.asarray(inputs["input"])[:, :-1].astype(np.int64)
    ts = np.asarray(inputs["input_timestamp"])[:, :-1].astype(np.int64)
    ei = np.asarray(inputs["edge_index"])

    blk_max = ts.reshape(B, L // STEP, STEP).max(axis=(0, 2))
    his_pos = np.repeat(np.clip(blk_max - 1, 0, T - 1), STEP)  # [200]

    active = sorted(set(int(t) for t in his_pos))
    sl = {}
    for t in active:
        dst = ei[t, 1].astype(np.int64)
        src = ei[t, 0].astype(np.int64)
        order = np.argsort(dst, kind="stable")
        ds, ss = dst[order], src[order]
        starts = np.searchsorted(ds, np.arange(N))
        ends = np.searchsorted(ds, np.arange(N), side="right")
        deg = (np.bincount(dst, minlength=N) + 1.0).astype(np.float32)
        norm = (1.0 / np.sqrt(deg)).astype(np.float32)
        sl[t] = (ss, starts, ends, norm)

    cores = []
    for c in range(NCORES):
        bsel = range(NBC * c, NBC * (c + 1))
        pair_id = {}
        pairs = []
        pos_pair = np.zeros((NBC, L), np.int64)
        for bi, b in enumerate(bsel):
            for l in range(L):
                key = (int(his_pos[l]), int(inp[b, l]))
                if key not in pair_id:
                    pair_id[key] = len(pairs)
                    pairs.append(key)
                pos_pair[bi, l] = pair_id[key]

        # L1 node set: pair nodes first, then 1-hop srcs
        l1_id = {}
        l1_nodes = []

        def _l1(t, n):
            if (t, n) not in l1_id:
                l1_id[(t, n)] = len(l1_nodes)
                l1_nodes.append((t, n))
            return l1_id[(t, n)]

        pair_edges = []
        for (t, n) in pairs:
            ss, st, en, norm = sl[t]
            srcs = ss[st[n]:en[n]]
            pair_edges.append((t, n, srcs))
            _l1(t, n)
            for s in srcs:
                _l1(t, int(s))

        # ---- pack L1 (aggregate emb_g into s1 at l1 nodes) ----
        pk1 = _Packer()
        l1_pid = np.zeros(len(l1_nodes), np.int64)
        for q, (t, n) in enumerate(l1_nodes):
            ss, st, en, norm = sl[t]
            srcs = np.concatenate([ss[st[n]:en[n]], [n]])      # + self
            l1_pid[q] = pk1.add(srcs, norm[srcs], norm[n])

        # ---- pack L2 (aggregate m into dyn at pair nodes) ----
        pk2 = _Packer()
        p2_pid = np.zeros(len(pairs), np.int64)
        for p, (t, n, srcs) in enumerate(pair_edges):
            ss, st, en, norm = sl[t]
            loc = np.array([l1_id[(t, int(s))] for s in srcs] + [l1_id[(t, n)]],
                           np.int64)
            cfs = np.concatenate([norm[srcs], [norm[n]]])
            p2_pid[p] = pk2.add(loc, cfs, norm[n])
        cores.append(dict(pk1=pk1, pk2=pk2, l1_pid=l1_pid, p2_pid=p2_pid,
                          pos_pair=pos_pair))

    def _ceil(x, m):
        return ((x + m - 1) // m) * m

    NSB1 = _ceil(max(c["pk1"].nbins + 1 for c in cores), CHUNK // BIN)
    NSB2 = _ceil(max(c["pk2"].nbins + 1 for c in cores), CHUNK // BIN)
    M1, P2 = NSB1 * 128, NSB2 * 128
    assert M1 <= 32000 and P2 <= 32000

    w12 = (inputs["gcn1_w"].astype(np.float32) @
           inputs["gcn2_w"].astype(np.float32))
    assert not np.any(inputs["gcn1_b"]), "gcn1_b folding requires zeros"

    pos_emb = np.asarray(inputs["pos_emb"], np.float32)
    posq = np.zeros((256, 8), np.float32)
    posq[:L] = pos_emb[:L]

    lcol = np.full((128, 2), 1e9, np.float32)
    lcol[:, 0] = np.arange(128)
    lcol[:72, 1] = np.arange(128, 200)

    out_w = np.asarray(inputs["out_w"], np.float32)
    scale_q = np.float32(1.0 / (np.sqrt(np.float32(DK)) + 1e-6))

    meta = dict(NSB1=NSB1, NSB2=NSB2,
                use_b2=bool(np.any(inputs["gcn2_b"])),
                use_lng=bool(np.any(inputs["ln_g"] != 1.0)),
                use_lnb=bool(np.any(inputs["ln_b"])),
                use_f1b=bool(np.any(inputs["ffn1_b"])),
                use_f2b=bool(np.any(inputs["ffn2_b"])),
                use_ob=bool(np.any(inputs["out_b"])),
                scale_q=float(scale_q))

    in_maps = []
    for c, cd in enumerate(cores):
        g1, d1, c1, nd1 = cd["pk1"].finish(NSB1)
        pk2 = cd["pk2"]
        l1_pid = cd["l1_pid"]
        pk2.src = [l1_pid[s] for s in pk2.src]
        g2, d2, c2, nd2 = pk2.finish(NSB2)

        dyidx = np.zeros((128, NBC * 16), np.int16)
        for bi in range(NBC):
            pid = cd["p2_pid"][cd["pos_pair"][bi]]
            dyidx[:, bi * 16:(bi + 1) * 16] = _wrap16(pid, 256)

        bsel = range(NBC * c, NBC * (c + 1))
        padmul = np.ones((128, 2 * NBC), np.float32)
        padadd = np.zeros((128, 2 * NBC), np.float32)
        for bi, b in enumerate(bsel):
            for qt in range(2):
                rows = QT_ROWS[qt]
                padv = (inp[b, qt * 128:qt * 128 + rows] == 0)
                padmul[:rows, 2 * bi + qt] = np.where(padv, 0.0, 1.0)
                padadd[:rows, 2 * bi + qt] = np.where(padv, NEG, 0.0)

        fpos = np.full((B, NSH), 1e9, np.float32)
        lo = c * NSH
        for b in range(B):
            u = inp[b]
            m = (u >= lo) & (u < lo + NSH)
            np.minimum.at(fpos[b], (u[m] - lo).astype(np.int64),
                          np.arange(L, dtype=np.float32)[m])
        pad0 = np.zeros((128, 2), np.float32)
        if lo == 0:
            # reference's tril-zeros mask col 0 for l<=198 only
            for qt in range(2):
                rows = QT_ROWS[qt]
                lv = qt * 128 + np.arange(rows)
                pad0[:rows, qt] = np.where(lv <= 198, -np.inf, 0.0)

        im = {
            "emb": np.asarray(inputs["emb_g"], np.float32),
            "l1_gidx": g1, "l1_dl": d1, "l1_cf": c1,
            "nrep1": np.tile(nd1, (64, 1)),
            "l2_gidx": g2, "l2_dl": d2, "l2_cf": c2,
            "nrep2": np.tile(nd2, (64, 1)),
            "w12": np.ascontiguousarray(w12),
            "dyidx": dyidx,
            "posq": posq,
            "wq": np.asarray(inputs["W_q"], np.float32),
            "wk": np.asarray(inputs["W_k"], np.float32),
            "wv": np.asarray(inputs["W_v"], np.float32),
            "wo": np.asarray(inputs["W_o"], np.float32),
            "f1w": np.asarray(inputs["ffn1_w"], np.float32),
            "f2w": np.asarray(inputs["ffn2_w"], np.float32),
            "padmul": padmul, "padadd": padadd, "lcol": lcol,
            "outwT": np.ascontiguousarray(out_w[lo:lo + NSH].T),
            "fpos": np.ascontiguousarray(fpos),
            "pad0": pad0,
        }
        if meta["use_b2"]:
            im["b2col"] = np.asarray(inputs["gcn2_b"], np.float32).reshape(64, 1)
        if meta["use_lng"]:
            im["lng"] = np.tile(np.asarray(inputs["ln_g"], np.float32), (128, 1))
        if meta["use_lnb"]:
            im["lnb"] = np.tile(np.asarray(inputs["ln_b"], np.float32), (128, 1))
        if meta["use_f1b"]:
            im["f1b"] = np.tile(np.asarray(inputs["ffn1_b"], np.float32), (128, 1))
        if meta["use_f2b"]:
            im["f2b"] = np.tile(np.asarray(inputs["ffn2_b"], np.float32), (128, 1))
        if meta["use_ob"]:
            im["obrep"] = np.tile(
                np.asarray(inputs["out_b"], np.float32)[lo:lo + NSH], (128, 1))
        in_maps.append(im)
    return meta, in_maps


# ----------------------------------------------------------------------
# device program
# ----------------------------------------------------------------------

def _build(meta, reps=1):
    NSB1, NSB2 = meta["NSB1"], meta["NSB2"]
    E1, M1 = NSB1 * BIN, NSB1 * 128
    E2, P2 = NSB2 * BIN, NSB2 * 128
    sq = meta["scale_q"]

    nc = bacc.Bacc("TRN2", target_bir_lowering=False, debug=False,
                   enable_asserts=True, num_devices=NCORES)

    def din(name, shape, dt=f32):
        return nc.dram_tensor(name, shape, dt, kind="ExternalInput")

    emb = din("emb", [N, D])
    g1 = din("l1_gidx", [128, E1 // 16], i16)
    d1 = din("l1_dl", [128, E1 // 128])
    c1 = din("l1_cf", [128, E1 // 128])
    nr1 = din("nrep1", [64, M1])
    g2 = din("l2_gidx", [128, E2 // 16], i16)
    d2 = din("l2_dl", [128, E2 // 128])
    c2 = din("l2_cf", [128, E2 // 128])
    nr2 = din("nrep2", [64, P2])
    w12 = din("w12", [64, 64])
    dyidx = din("dyidx", [128, NBC * 16], i16)
    posq = din("posq", [256, 8])
    wq, wk, wv = din("wq", [72, 512]), din("wk", [72, 512]), din("wv", [72, 512])
    wo = din("wo", [512, 72])
    f1w, f2w = din("f1w", [72, 72]), din("f2w", [72, 72])
    padmul_d = din("padmul", [128, 2 * NBC])
    padadd_d = din("padadd", [128, 2 * NBC])
    lcol_d = din("lcol", [128, 2])
    outwT = din("outwT", [72, NSH])
    fpos = din("fpos", [B, NSH])
    pad0 = din("pad0", [128, 2])
    b2col = din("b2col", [64, 1]) if meta["use_b2"] else None
    lng = din("lng", [128, 72]) if meta["use_lng"] else None
    lnb = din("lnb", [128, 72]) if meta["use_lnb"] else None
    f1b = din("f1b", [128, 72]) if meta["use_f1b"] else None
    f2b = din("f2b", [128, 72]) if meta["use_f2b"] else None
    obrep = din("obrep", [128, NSH]) if meta["use_ob"] else None

    phases = os.environ.get("KPHASES", "gcn,tf,lg").split(",")
    small = os.environ.get("KSMALL") == "1"
    assert not (small and "lg" in phases)
    out = nc.dram_tensor("out", [128, 64] if small else [B * L, NSH], f32,
                         kind="ExternalOutput")

    with tile.TileContext(nc) as tc:
      for _rep in range(reps):
       with ExitStack() as es:
        cst = es.enter_context(tc.tile_pool(name="cst", bufs=1))
        glob = es.enter_context(tc.tile_pool(name="glob", bufs=1))
        dramp = es.enter_context(tc.tile_pool(name="dramp", bufs=1,
                                              space="DRAM"))

        # ---------- constants ----------
        iota_i = cst.tile([128, NBLK, 128], i32)
        nc.gpsimd.iota(iota_i[:], [[0, NBLK], [1, 128]], base=0,
                       channel_multiplier=0)
        iota_f = cst.tile([128, NBLK, 128], f32)
        nc.vector.tensor_copy(iota_f[:], iota_i[:])
        ident = cst.tile([128, 128], f32)
        make_identity(nc, ident[:])
        zer200 = cst.tile([128, 200], f32)
        nc.gpsimd.memset(zer200[:], 0.0)
        causal = cst.tile([128, 2, 200], f32)
        for qt in range(2):
            nc.gpsimd.affine_select(
                out=causal[:, qt, :], in_=zer200[:], pattern=[[-1, 200]],
                compare_op=mybir.AluOpType.is_ge, fill=NEG,
                base=128 * qt, channel_multiplier=1)

        lcol_t = cst.tile([128, 2], f32)
        nc.sync.dma_start(lcol_t[:], lcol_d[:, :])
        pad0_t = cst.tile([128, 2], f32)
        nc.sync.dma_start(pad0_t[:], pad0[:, :])
        padmul_t = cst.tile([128, 2 * NBC], f32)
        nc.sync.dma_start(padmul_t[:], padmul_d[:, :])
        padadd_t = cst.tile([128, 2 * NBC], f32)
        nc.sync.dma_start(padadd_t[:], padadd_d[:, :])

        wq_t = glob.tile([72, 512], f32)
        nc.sync.dma_start(wq_t[:], wq[:, :])
        wk_t = glob.tile([72, 512], f32)
        nc.sync.dma_start(wk_t[:], wk[:, :])
        wv_t = glob.tile([72, 512], f32)
        nc.sync.dma_start(wv_t[:], wv[:, :])
        wo_t = glob.tile([128, 4, 72], f32)
        nc.sync.dma_start(wo_t[:], wo.ap().rearrange("(k p) d -> p k d", p=128))
        f1w_t = glob.tile([72, 72], f32)
        nc.sync.dma_start(f1w_t[:], f1w[:, :])
        f2w_t = glob.tile([72, 72], f32)
        nc.sync.dma_start(f2w_t[:], f2w[:, :])
        w12_t = glob.tile([64, 64], f32)
        nc.sync.dma_start(w12_t[:], w12[:, :])
        opt = {}
        for nm, dd, shp in (("lng", lng, [128, 72]), ("lnb", lnb, [128, 72]),
                            ("f1b", f1b, [128, 72]), ("f2b", f2b, [128, 72]),
                            ("b2col", b2col, [64, 1])):
            if dd is not None:
                tt = glob.tile(shp, f32)
                nc.sync.dma_start(tt[:], dd[:, :])
                opt[nm] = tt

        x2loc = dramp.tile([72, NBC * L], f32)
        x2g = dramp.tile([NCORES * 72, NBC * L], f32)
        m_dram = dramp.tile([M1, 64], f32)
        dyn_dram = dramp.tile([P2, 64], f32)
        x2full = glob.tile([72, B * L], f32)

        # ================= GCN =================
        if "gcn" in phases:
         with (
            tc.tile_pool(name="gcn", bufs=1) as gcn,
            tc.tile_pool(name="msgp", bufs=2) as msgp,
            tc.tile_pool(name="ohp", bufs=2) as ohp,
            tc.tile_pool(name="gps", bufs=3, space="PSUM") as gps,
            tc.tile_pool(name="mps", bufs=3, space="PSUM") as mps,
         ):
            s1T = gcn.tile([64, M1], f32)

            def seg_sum(gsrc_ap, gidx_d, dl_d, cf_d, nsb, outT, nrep_t, extra,
                        tagp):
                gi = gcn.tile([128, (nsb * BIN) // 16], i16, tag="gi" + tagp)
                nc.sync.dma_start(gi[:], gidx_d[:, :])
                dlt = gcn.tile([128, nsb * NBLK], f32, tag="dl" + tagp)
                nc.sync.dma_start(dlt[:], dl_d[:, :])
                cft = gcn.tile([128, nsb * NBLK], f32, tag="cf" + tagp)
                nc.sync.dma_start(cft[:], cf_d[:, :])
                nch = (nsb * BIN) // CHUNK
                bpc = CHUNK // BIN
                for ch in range(nch):
                    msg = msgp.tile([128, CHUNK // 128, 64], f32, tag="msg")
                    nc.gpsimd.dma_gather(
                        msg[:], gsrc_ap,
                        gi[:, ch * (CHUNK // 16):(ch + 1) * (CHUNK // 16)],
                        CHUNK, CHUNK, 64, single_packet=False)
                    for sb in range(bpc):
                        g = ch * bpc + sb
                        oh = ohp.tile([128, NBLK, 128], f32, tag="oh")
                        bsl = slice(g * NBLK, (g + 1) * NBLK)
                        nc.vector.tensor_tensor(
                            out=oh[:], in0=iota_f[:],
                            in1=dlt[:, bsl].to_broadcast([128, NBLK, 128]),
                            op=mybir.AluOpType.is_equal)
                        nc.vector.tensor_tensor(
                            out=oh[:], in0=oh[:],
                            in1=cft[:, bsl].to_broadcast([128, NBLK, 128]),
                            op=mybir.AluOpType.mult)
                        acc = gps.tile([64, 128], f32, space="PSUM", tag="acc")
                        for k in range(NBLK):
                            nc.tensor.matmul(acc[:, :],
                                             msg[:, sb * NBLK + k, :],
                                             oh[:, k, :],
                                             start=(k == 0),
                                             stop=(k == NBLK - 1))
                        sl = outT[:, g * 128:(g + 1) * 128]
                        nc.vector.tensor_tensor(
                            out=sl, in0=acc[:, :],
                            in1=nrep_t[:, g * 128:(g + 1) * 128],
                            op=mybir.AluOpType.mult)
                        if extra is not None:
                            extra(sl)

            nr1_t = gcn.tile([64, M1], f32)
            nc.sync.dma_start(nr1_t[:], nr1[:, :])
            seg_sum(emb[:, :], g1, d1, c1, NSB1, s1T, nr1_t, None, "1")

            # m = s1 @ w12, row-major to DRAM (batched copies + one DMA)
            m_sb = gcn.tile([128, M1 // 128, 64], f32)
            for j in range(M1 // 128):
                pm = mps.tile([128, 64], f32, space="PSUM", tag="pm")
                nc.tensor.matmul(pm[:, :], s1T[:, j * 128:(j + 1) * 128],
                                 w12_t[:], start=True, stop=True)
                nc.scalar.copy(m_sb[:, j, :], pm[:, :])
            nc.sync.dma_start(m_dram[:].rearrange("(c p) d -> p c d", p=128),
                              m_sb[:])

            # L2: m -> dynT
            dynT = gcn.tile([64, P2], f32)

            def add_b2(sl):
                if "b2col" in opt:
                    nc.vector.tensor_scalar(sl, sl, opt["b2col"][:, 0:1], None,
                                            op0=mybir.AluOpType.add)

            nr2_t = gcn.tile([64, P2], f32)
            nc.sync.dma_start(nr2_t[:], nr2[:, :])
            seg_sum(m_dram[:, :], g2, d2, c2, NSB2, dynT, nr2_t,
                    add_b2 if meta["use_b2"] else None, "2")

            # dyn rows to DRAM
            dyn_sb = gcn.tile([128, P2 // 128, 64], f32)
            for j in range(P2 // 128):
                pt = mps.tile([128, 64], f32, space="PSUM", tag="pm")
                nc.tensor.transpose(pt[:, 0:64], dynT[:, j * 128:(j + 1) * 128],
                                    ident[0:64, 0:64])
                nc.scalar.copy(dyn_sb[:, j, :], pt[:, 0:64])
            nc.sync.dma_start(dyn_dram[:].rearrange("(c p) d -> p c d", p=128),
                              dyn_sb[:])

        # ================= transformer (2 batches) =================
        if "tf" in phases:
         with (
            tc.tile_pool(name="tf", bufs=1) as tf,
            tc.tile_pool(name="tfw", bufs=2) as tfw,
            tc.tile_pool(name="psa", bufs=3, space="PSUM") as psa,
            tc.tile_pool(name="psb", bufs=3, space="PSUM") as psb,
         ):
            dyidx_t = tf.tile([128, NBC * 16], i16)
            nc.sync.dma_start(dyidx_t[:], dyidx[:, :])
            for bi in range(NBC):
                x_row = tfw.tile([128, 2, 72], f32, tag="x_row")
                nc.vector.memset(x_row[:], 0.0)
                dy = tfw.tile([128, 2, 64], f32, tag="dy")
                nc.gpsimd.memset(dy[:], 0.0)
                nc.gpsimd.dma_gather(dy[:], dyn_dram[:, :],
                                     dyidx_t[:, bi * 16:(bi + 1) * 16],
                                     256, 200, 64, single_packet=False)
                nc.vector.tensor_copy(x_row[:, :, 0:64], dy[:])
                nc.sync.dma_start(x_row[:, :, 64:72],
                                  posq.ap().rearrange("(c p) d -> p c d",
                                                      p=128))

                def transpose72(src_row, dst_T):
                    for cblk in range(2):
                        pt = psb.tile([128, 128], f32, space="PSUM", tag="ptr")
                        nc.tensor.transpose(pt[0:72, :], src_row[:, cblk, :],
                                            ident[:])
                        nc.scalar.copy(dst_T[:, cblk * 128:(cblk + 1) * 128],
                                       pt[0:72, :])

                x_T = tfw.tile([72, 256], f32, tag="x_T")
                transpose72(x_row, x_T)

                # QKV
                q_T = tfw.tile([128, 4, 200], f32, tag="q_T")
                k_T = tfw.tile([128, 4, 200], f32, tag="k_T")
                for j in range(4):
                    pq = psa.tile([128, 200], f32, space="PSUM", tag="ps")
                    nc.tensor.matmul(pq[:, :], wq_t[:, j * 128:(j + 1) * 128],
                                     x_T[:, 0:200], start=True, stop=True)
                    nc.scalar.mul(q_T[:, j, :], pq[:, :], sq)
                    pk = psa.tile([128, 200], f32, space="PSUM", tag="ps")
                    nc.tensor.matmul(pk[:, :], wk_t[:, j * 128:(j + 1) * 128],
                                     x_T[:, 0:200], start=True, stop=True)
                    nc.scalar.copy(k_T[:, j, :], pk[:, :])
                v_row = tfw.tile([128, 2, 512], f32, tag="v_row")
                pv = psa.tile([128, 512], f32, space="PSUM", tag="ps")
                nc.tensor.matmul(pv[:, :], x_T[:, 0:128], wv_t[:],
                                 start=True, stop=True)
                nc.scalar.copy(v_row[:, 0, :], pv[:, :])
                pv2 = psa.tile([128, 512], f32, space="PSUM", tag="ps")
                nc.tensor.matmul(pv2[0:72, :], x_T[:, 128:200], wv_t[:],
                                 start=True, stop=True)
                nc.scalar.copy(v_row[0:72, 1, :], pv2[0:72, :])

                # attention
                at_T = tfw.tile([128, 2, 8, 200], f32, tag="at_T")
                vat_T = tfw.tile([128, 4, 200], f32, tag="vat_T")
                for qt in range(2):
                    qr = QT_ROWS[qt]
                    qsl = slice(qt * 128, qt * 128 + qr)
                    comb = tfw.tile([128, 200], f32, tag="comb")
                    nc.vector.tensor_scalar(
                        comb[:], causal[:, qt, :],
                        padadd_t[:, 2 * bi + qt:2 * bi + qt + 1], None,
                        op0=mybir.AluOpType.min)
                    sc = tfw.tile([128, 8, 200], f32, tag="sc")
                    nc.gpsimd.memset(sc[:], 0.0)
                    for h in range(8):
                        hr = slice((h % 2) * 64, (h % 2) * 64 + 64)
                        ps_sc = psa.tile([128, 200], f32, space="PSUM",
                                         tag="ps")
                        nc.tensor.matmul(ps_sc[0:qr, :], q_T[hr, h // 2, qsl],
                                         k_T[hr, h // 2, 0:200],
                                         start=True, stop=True)
                        nc.vector.scalar_tensor_tensor(
                            out=sc[0:qr, h, :], in0=ps_sc[0:qr, :],
                            scalar=padmul_t[0:qr, 2 * bi + qt:2 * bi + qt + 1],
                            in1=comb[0:qr, :],
                            op0=mybir.AluOpType.mult, op1=mybir.AluOpType.add)
                    nmax = tfw.tile([128, 8], f32, tag="nmax")
                    nc.vector.tensor_reduce(nmax[0:qr, :], sc[0:qr, :, :],
                                            axis=mybir.AxisListType.X,
                                            op=mybir.AluOpType.max, negate=True)
                    rs = tfw.tile([128, 8], f32, tag="rs")
                    for h in range(8):
                        nc.scalar.activation(
                            sc[0:qr, h, :], sc[0:qr, h, :],
                            mybir.ActivationFunctionType.Exp,
                            bias=nmax[0:qr, h:h + 1], scale=1.0,
                            accum_out=rs[0:qr, h:h + 1])
                    rr = tfw.tile([128, 8], f32, tag="rr")
                    nc.vector.reciprocal(rr[0:qr, :], rs[0:qr, :])
                    for h in range(8):
                        nc.vector.tensor_scalar_mul(sc[0:qr, h, :],
                                                    sc[0:qr, h, :],
                                                    rr[0:qr, h:h + 1])
                    for h in range(8):
                        for kt in range(2):
                            ks = QT_ROWS[kt]
                            pt = psb.tile([128, 128], f32, space="PSUM",
                                          tag="ptr")
                            nc.tensor.transpose(
                                pt[0:ks, 0:qr],
                                sc[0:qr, h, kt * 128:kt * 128 + ks],
                                ident[0:qr, 0:qr])
                            nc.scalar.copy(at_T[0:ks, kt, h, qsl],
                                           pt[0:ks, 0:qr])
                for h in range(8):
                    hr = slice((h % 2) * 64, (h % 2) * 64 + 64)
                    for qt in range(2):
                        qr = QT_ROWS[qt]
                        qsl = slice(qt * 128, qt * 128 + qr)
                        pvt = psa.tile([64, 128], f32, space="PSUM", tag="ps")
                        for kt in range(2):
                            ks = QT_ROWS[kt]
                            nc.tensor.matmul(
                                pvt[:, 0:qr],
                                v_row[0:ks, kt, h * 64:(h + 1) * 64],
                                at_T[0:ks, kt, h, qsl],
                                start=(kt == 0), stop=(kt == 1))
                        nc.scalar.copy(vat_T[hr, h // 2, qsl], pvt[:, 0:qr])

                # att_out + x -> LN -> x1
                y = tfw.tile([128, 2, 72], f32, tag="y")
                nc.gpsimd.memset(y[:], 0.0)
                for qt in range(2):
                    qr = QT_ROWS[qt]
                    qsl = slice(qt * 128, qt * 128 + qr)
                    pat = psa.tile([128, 72], f32, space="PSUM", tag="ps")
                    for kb in range(4):
                        nc.tensor.matmul(pat[0:qr, :], vat_T[:, kb, qsl],
                                         wo_t[:, kb, :],
                                         start=(kb == 0), stop=(kb == 3))
                    nc.vector.tensor_tensor(out=y[0:qr, qt, :],
                                            in0=x_row[0:qr, qt, :],
                                            in1=pat[0:qr, :],
                                            op=mybir.AluOpType.add)

                def layer_norm(y_t, x1_t):
                    sums = tfw.tile([128, 2], f32, tag="lnsum")
                    nc.vector.tensor_reduce(sums[:], y_t[:, :, :],
                                            axis=mybir.AxisListType.X,
                                            op=mybir.AluOpType.add)
                    mu = tfw.tile([128, 2], f32, tag="lnmu")
                    nc.vector.tensor_scalar_mul(mu[:], sums[:], 1.0 / 72.0)
                    vs = tfw.tile([128, 2], f32, tag="lnvs")
                    scr = tfw.tile([128, 2, 72], f32, tag="lnscr")
                    for qt in range(2):
                        nc.vector.tensor_scalar(
                            x1_t[:, qt, :], y_t[:, qt, :], mu[:, qt:qt + 1],
                            None, op0=mybir.AluOpType.subtract)
                        nc.scalar.activation(
                            scr[:, qt, :], x1_t[:, qt, :],
                            mybir.ActivationFunctionType.Square,
                            accum_out=vs[:, qt:qt + 1])
                    sd = tfw.tile([128, 2], f32, tag="lnsd")
                    nc.vector.tensor_scalar(sd[:], vs[:], 1.0 / 72.0, 1e-5,
                                            op0=mybir.AluOpType.mult,
                                            op1=mybir.AluOpType.add)
                    nc.scalar.activation(sd[:], sd[:],
                                         mybir.ActivationFunctionType.Sqrt)
                    rstd = tfw.tile([128, 2], f32, tag="lnrstd")
                    nc.vector.reciprocal(rstd[:], sd[:])
                    for qt in range(2):
                        nc.vector.tensor_scalar_mul(x1_t[:, qt, :],
                                                    x1_t[:, qt, :],
                                                    rstd[:, qt:qt + 1])
                        if "lng" in opt:
                            nc.vector.tensor_tensor(
                                out=x1_t[:, qt, :], in0=x1_t[:, qt, :],
                                in1=opt["lng"][:, :], op=mybir.AluOpType.mult)
                        if "lnb" in opt:
                            nc.vector.tensor_tensor(
                                out=x1_t[:, qt, :], in0=x1_t[:, qt, :],
                                in1=opt["lnb"][:, :], op=mybir.AluOpType.add)

                x1_row = tfw.tile([128, 2, 72], f32, tag="x1_row")
                layer_norm(y, x1_row)

                x1_T = tfw.tile([72, 256], f32, tag="x1_T")
                transpose72(x1_row, x1_T)
                f1_row = tfw.tile([128, 2, 72], f32, tag="f1_row")
                nc.gpsimd.memset(f1_row[:], 0.0)
                for qt in range(2):
                    qr = QT_ROWS[qt]
                    qsl = slice(qt * 128, qt * 128 + qr)
                    pf = psa.tile([128, 72], f32, space="PSUM", tag="ps")
                    nc.tensor.matmul(pf[0:qr, :], x1_T[:, qsl], f1w_t[:],
                                     start=True, stop=True)
                    if "f1b" in opt:
                        nc.vector.tensor_tensor(out=pf[0:qr, :],
                                                in0=pf[0:qr, :],
                                                in1=opt["f1b"][0:qr, :],
                                                op=mybir.AluOpType.add)
                    nc.scalar.activation(f1_row[0:qr, qt, :], pf[0:qr, :],
                                         mybir.ActivationFunctionType.Relu)
                f1_T = tfw.tile([72, 256], f32, tag="f1_T")
                transpose72(f1_row, f1_T)
                y2 = tfw.tile([128, 2, 72], f32, tag="y2")
                nc.gpsimd.memset(y2[:], 0.0)
                for qt in range(2):
                    qr = QT_ROWS[qt]
                    qsl = slice(qt * 128, qt * 128 + qr)
                    pf2 = psa.tile([128, 72], f32, space="PSUM", tag="ps")
                    nc.tensor.matmul(pf2[0:qr, :], f1_T[:, qsl], f2w_t[:],
                                     start=True, stop=True)
                    if "f2b" in opt:
                        nc.vector.tensor_tensor(out=pf2[0:qr, :],
                                                in0=pf2[0:qr, :],
                                                in1=opt["f2b"][0:qr, :],
                                                op=mybir.AluOpType.add)
                    nc.vector.tensor_tensor(out=y2[0:qr, qt, :],
                                            in0=x1_row[0:qr, qt, :],
                                            in1=pf2[0:qr, :],
                                            op=mybir.AluOpType.add)
                x2_row = tfw.tile([128, 2, 72], f32, tag="x2_row")
                layer_norm(y2, x2_row)
                x2_T = tfw.tile([72, 256], f32, tag="x2_T")
                transpose72(x2_row, x2_T)
                nc.sync.dma_start(x2loc[:, bi * L:(bi + 1) * L], x2_T[:, 0:200])

            nc.gpsimd.collective_compute(
                "AllGather", mybir.AluOpType.bypass,
                replica_groups=[list(range(NCORES))],
                ins=[x2loc[:].opt()], outs=[x2g[:].opt()])
            for cc in range(NCORES):
                nc.sync.dma_start(
                    x2full[:, cc * NBC * L:(cc + 1) * NBC * L],
                    x2g[cc * 72:(cc + 1) * 72, :])

        # ================= logits + previous-user mask =================
        if "lg" in phases:
         with (
            tc.tile_pool(name="lg", bufs=2) as lg,
            tc.tile_pool(name="lgc", bufs=1) as lgc,
            tc.tile_pool(name="pslg", bufs=6, space="PSUM") as pslg,
         ):
            outwT_t = lgc.tile([72, NSH], f32)
            nc.sync.dma_start(outwT_t[:], outwT[:, :])
            negf = lgc.tile([128, NSH], f32)
            nc.gpsimd.memset(negf[:], float("-inf"))
            ob_t = None
            if obrep is not None:
                ob_t = lgc.tile([128, NSH], f32)
                nc.sync.dma_start(ob_t[:], obrep[:, :])
            for b in range(B):
                fp_rep = lg.tile([128, NSH], f32, tag="fp")
                nc.sync.dma_start(fp_rep[:],
                                  fpos[b:b + 1, :].to_broadcast([128, NSH]))
                for qt in range(2):
                    qr = QT_ROWS[qt]
                    msk = lg.tile([128, NSH], i8, tag="msk")
                    nc.vector.tensor_scalar(
                        msk[0:qr, :], fp_rep[0:qr, :],
                        lcol_t[0:qr, qt:qt + 1], None,
                        op0=mybir.AluOpType.is_le)
                    ot = lg.tile([128, NSH], f32, tag="ot")
                    for (v0, w) in VT:
                        plg = pslg.tile([128, 512], f32, space="PSUM",
                                        tag="plg")
                        nc.tensor.matmul(
                            plg[0:qr, 0:w],
                            x2full[:, b * L + qt * 128: b * L + qt * 128 + qr],
                            outwT_t[:, v0:v0 + w], start=True, stop=True)
                        if ob_t is not None:
                            nc.vector.tensor_tensor(
                                out=ot[0:qr, v0:v0 + w], in0=plg[0:qr, 0:w],
                                in1=ob_t[0:qr, v0:v0 + w],
                                op=mybir.AluOpType.add)
                        else:
                            nc.scalar.copy(ot[0:qr, v0:v0 + w],
                                           plg[0:qr, 0:w])
                    nc.vector.copy_predicated(ot[0:qr, :], msk[0:qr, :],
                                              negf[0:qr, :])
                    nc.vector.tensor_tensor(
                        out=ot[0:qr, 0:1], in0=ot[0:qr, 0:1],
                        in1=pad0_t[0:qr, qt:qt + 1], op=mybir.AluOpType.add)
                    r0 = b * L + qt * 128
                    nc.sync.dma_start(out[r0:r0 + qr, :], ot[0:qr, :])

    nc.compile()
    return nc


# ----------------------------------------------------------------------
# entry point
# ----------------------------------------------------------------------

def kernel(**inputs):
    meta, in_maps = _host_prep(inputs)
    key = tuple(sorted(meta.items()))
    if key not in _CACHE:
        _CACHE[key] = _build(meta)
    nc = _CACHE[key]
    res = bass_utils.run_bass_kernel_spmd(nc, in_maps,
                                          core_ids=list(range(NCORES)))
    return np.concatenate([res.results[c]["out"] for c in range(NCORES)],
                          axis=1)


# revision 20
# speedup vs baseline: 1.1747x; 1.1747x over previous
"""DyHGCN_H forward as a Trainium2 Bass kernel, SPMD over 8 NeuronCores.

Sharding:
  - GCN + transformer: data-parallel over batch (2 sequences per core). The
    GCN is evaluated sparsely: only the (time-slice, node) pairs actually
    gathered by dyemb are computed, via their 2-hop in-neighborhoods.
  - Aggregation is a gather + one-hot matmul segment-sum (edges sorted by
    destination, packed in 2048-edge bins, accumulated in PSUM).
  - x2 is all-gathered (72 x 3200), then the [30000, 72] output projection +
    previous-user mask run tensor-parallel over the vocab axis (3750/core).
"""
import os
import sys
from contextlib import ExitStack

for _p in ("/opt/trn_rl_repo", "/root/.axon_site/_ro/trn_rl_repo"):
    if os.path.isdir(_p) and _p not in sys.path:
        sys.path.insert(0, _p)

import numpy as np
from concourse import bacc, mybir, tile
from concourse import bass_utils
from concourse.masks import make_identity

f32 = mybir.dt.float32
i16 = mybir.dt.int16
i32 = mybir.dt.int32
i8 = mybir.dt.int8

B, L1TOT, N, D, T = 16, 201, 30000, 64, 8
L, INP, H, DK = 200, 72, 8, 64
STEP = 5
NCORES, NBC = 8, 2          # cores, batches per core
NSH = N // NCORES           # vocab shard: 3750
NEG = float(-2 ** 32 + 1)
BIN = 2048                  # edges per superblock (one 128-wide PSUM window)
NBLK = BIN // 128           # matmul blocks per bin
CHUNK = 8192                # edges per dma_gather call
QT_ROWS = (128, 72)         # rows per q-tile (200 = 128 + 72)
VT = [(v, min(512, NSH - v)) for v in range(0, NSH, 512)]  # vocab tiles

_CACHE = {}


# ----------------------------------------------------------------------
# host-side index packing
# ----------------------------------------------------------------------

def _wrap16(idx, cap):
    """idx j -> [16, cap/16] at (j%16, j//16), tiled to 128 partitions."""
    buf = np.full(cap, -1, np.int16)
    buf[: len(idx)] = np.asarray(idx, np.int16)
    return np.tile(buf.reshape(cap // 16, 16).T, (8, 1))


def _wrap128(vals, cap, fill):
    v = np.full(cap, fill, np.float32)
    v[: len(vals)] = vals
    return np.ascontiguousarray(v.reshape(cap // 128, 128).T)


class _Packer:
    """Pack per-destination edge groups into BIN-edge superblocks, each
    covering <=128 distinct destinations; assigns padded local ids."""

    def __init__(self):
        self.src = []
        self.dl = []
        self.cf = []
        self.norm_d = []       # per padded id (len nbins*128)
        self.nbins = 0
        self._used_e = 0
        self._used_d = 0

    def _flush(self):
        if self._used_e or self._used_d:
            pad = BIN - self._used_e
            if pad:
                self.src.append(np.zeros(pad, np.int64))
                self.dl.append(np.full(pad, -1.0, np.float32))
                self.cf.append(np.zeros(pad, np.float32))
            self.norm_d.extend([0.0] * (128 - self._used_d))
            self.nbins += 1
            self._used_e = 0
            self._used_d = 0

    def add(self, srcs, cfs, norm_dst):
        k = len(srcs)
        assert 0 < k <= BIN, k
        if self._used_e + k > BIN or self._used_d >= 128:
            self._flush()
        pid = self.nbins * 128 + self._used_d
        self.src.append(np.asarray(srcs, np.int64))
        self.cf.append(np.asarray(cfs, np.float32))
        self.dl.append(np.full(k, float(self._used_d), np.float32))
        self.norm_d.append(float(norm_dst))
        self._used_e += k
        self._used_d += 1
        return pid

    def finish(self, nbins_cap):
        self._flush()
        assert self.nbins <= nbins_cap, (self.nbins, nbins_cap)
        e_cap = nbins_cap * BIN
        src = np.concatenate(self.src) if self.src else np.zeros(0, np.int64)
        dl = np.concatenate(self.dl) if self.dl else np.zeros(0, np.float32)
        cf = np.concatenate(self.cf) if self.cf else np.zeros(0, np.float32)
        nd = np.zeros(nbins_cap * 128, np.float32)
        nd[: len(self.norm_d)] = self.norm_d
        src_full = np.zeros(e_cap, np.int64)   # pad with valid idx 0
        src_full[: len(src)] = src
        return (_wrap16(src_full, e_cap), _wrap128(dl, e_cap, -1.0),
                _wrap128(cf, e_cap, 0.0), nd)


def _host_prep(inputs):
    inp = np.asarray(inputs["input"])[:, :-1].astype(np.int64)
    ts = np.asarray(inputs["input_timestamp"])[:, :-1].astype(np.int64)
    ei = np.asarray(inputs["edge_index"])

    blk_max = ts.reshape(B, L // STEP, STEP).max(axis=(0, 2))
    his_pos = np.repeat(np.clip(blk_max - 1, 0, T - 1), STEP)  # [200]

    active = sorted(set(int(t) for t in his_pos))
    sl = {}
    for t in active:
        dst = ei[t, 1].astype(np.int64)
        src = ei[t, 0].astype(np.int64)
        order = np.argsort(dst, kind="stable")
        ds, ss = dst[order], src[order]
        starts = np.searchsorted(ds, np.arange(N))
        ends = np.searchsorted(ds, np.arange(N), side="right")
        deg = (np.bincount(dst, minlength=N) + 1.0).astype(np.float32)
        norm = (1.0 / np.sqrt(deg)).astype(np.float32)
        sl[t] = (ss, starts, ends, norm)

    # ---- global L1 node set (deduped across cores), sharded round-robin ----
    l1_id = {}
    l1_nodes = []

    def _l1(t, n):
        if (t, n) not in l1_id:
            l1_id[(t, n)] = len(l1_nodes)
            l1_nodes.append((t, n))
        return l1_id[(t, n)]

    cores = []
    for c in range(NCORES):
        bsel = range(NBC * c, NBC * (c + 1))
        pair_id = {}
        pairs = []
        pos_pair = np.zeros((NBC, L), np.int64)
        for bi, b in enumerate(bsel):
            for l in range(L):
                key = (int(his_pos[l]), int(inp[b, l]))
                if key not in pair_id:
                    pair_id[key] = len(pairs)
                    pairs.append(key)
                pos_pair[bi, l] = pair_id[key]
        pair_edges = []
        for (t, n) in pairs:
            ss, st, en, norm = sl[t]
            srcs = ss[st[n]:en[n]]
            pair_edges.append((t, n, srcs))
            _l1(t, n)
            for s in srcs:
                _l1(t, int(s))
        cores.append(dict(pairs=pairs, pair_edges=pair_edges,
                          pos_pair=pos_pair))

    # pack each core's shard of the global L1 set
    l1_pid_glob = np.zeros(len(l1_nodes), np.int64)   # -> (core, local pid)
    l1_core = np.zeros(len(l1_nodes), np.int64)
    pk1s = [_Packer() for _ in range(NCORES)]
    for q, (t, n) in enumerate(l1_nodes):
        c = q % NCORES
        ss, st, en, norm = sl[t]
        srcs = np.concatenate([ss[st[n]:en[n]], [n]])          # + self
        l1_pid_glob[q] = pk1s[c].add(srcs, norm[srcs], norm[n])
        l1_core[q] = c

    # ---- pack L2 per core (srcs reference the global gathered m table) ----
    for c in range(NCORES):
        cd = cores[c]
        pk2 = _Packer()
        p2_pid = np.zeros(len(cd["pairs"]), np.int64)
        for p, (t, n, srcs) in enumerate(cd["pair_edges"]):
            ss, st, en, norm = sl[t]
            loc = np.array([l1_id[(t, int(s))] for s in srcs] +
                           [l1_id[(t, n)]], np.int64)
            cfs = np.concatenate([norm[srcs], [norm[n]]])
            p2_pid[p] = pk2.add(loc, cfs, norm[n])   # loc -> global pid later
        cd["pk2"] = pk2
        cd["p2_pid"] = p2_pid

    def _ceil(x, m):
        return ((x + m - 1) // m) * m

    NSB1 = _ceil(max(pk.nbins + 1 for pk in pk1s), CHUNK // BIN)
    NSB2 = _ceil(max(c["pk2"].nbins + 1 for c in cores), CHUNK // BIN)
    M1, P2 = NSB1 * 128, NSB2 * 128
    assert NCORES * M1 <= 32000 and P2 <= 32000
    # global padded id for each dense l1 id
    l1_glob = l1_core * M1 + l1_pid_glob

    w12 = (inputs["gcn1_w"].astype(np.float32) @
           inputs["gcn2_w"].astype(np.float32))
    assert not np.any(inputs["gcn1_b"]), "gcn1_b folding requires zeros"

    pos_emb = np.asarray(inputs["pos_emb"], np.float32)
    posq = np.zeros((256, 8), np.float32)
    posq[:L] = pos_emb[:L]

    lcol = np.full((128, 2), 1e9, np.float32)
    lcol[:, 0] = np.arange(128)
    lcol[:72, 1] = np.arange(128, 200)

    out_w = np.asarray(inputs["out_w"], np.float32)
    scale_q = np.float32(1.0 / (np.sqrt(np.float32(DK)) + 1e-6))

    meta = dict(NSB1=NSB1, NSB2=NSB2,
                use_b2=bool(np.any(inputs["gcn2_b"])),
                use_lng=bool(np.any(inputs["ln_g"] != 1.0)),
                use_lnb=bool(np.any(inputs["ln_b"])),
                use_f1b=bool(np.any(inputs["ffn1_b"])),
                use_f2b=bool(np.any(inputs["ffn2_b"])),
                use_ob=bool(np.any(inputs["out_b"])),
                scale_q=float(scale_q))

    in_maps = []
    for c, cd in enumerate(cores):
        g1, d1, c1, nd1 = pk1s[c].finish(NSB1)
        pk2 = cd["pk2"]
        pk2.src = [l1_glob[s] for s in pk2.src]
        g2, d2, c2, nd2 = pk2.finish(NSB2)

        dyidx = np.zeros((128, NBC * 16), np.int16)
        for bi in range(NBC):
            pid = cd["p2_pid"][cd["pos_pair"][bi]]
            dyidx[:, bi * 16:(bi + 1) * 16] = _wrap16(pid, 256)

        bsel = range(NBC * c, NBC * (c + 1))
        padmul = np.ones((128, 2 * NBC), np.float32)
        padadd = np.zeros((128, 2 * NBC), np.float32)
        for bi, b in enumerate(bsel):
            for qt in range(2):
                rows = QT_ROWS[qt]
                padv = (inp[b, qt * 128:qt * 128 + rows] == 0)
                padmul[:rows, 2 * bi + qt] = np.where(padv, 0.0, 1.0)
                padadd[:rows, 2 * bi + qt] = np.where(padv, NEG, 0.0)

        fpos = np.full((B, NSH), 1e9, np.float32)
        lo = c * NSH
        for b in range(B):
            u = inp[b]
            m = (u >= lo) & (u < lo + NSH)
            np.minimum.at(fpos[b], (u[m] - lo).astype(np.int64),
                          np.arange(L, dtype=np.float32)[m])
        pad0 = np.zeros((128, 2), np.float32)
        if lo == 0:
            # reference's tril-zeros mask col 0 for l<=198 only
            for qt in range(2):
                rows = QT_ROWS[qt]
                lv = qt * 128 + np.arange(rows)
                pad0[:rows, qt] = np.where(lv <= 198, -np.inf, 0.0)

        im = {
            "emb": np.asarray(inputs["emb_g"], np.float32),
            "l1_gidx": g1, "l1_dl": d1, "l1_cf": c1,
            "nrep1": np.tile(nd1, (64, 1)),
            "l2_gidx": g2, "l2_dl": d2, "l2_cf": c2,
            "nrep2": np.tile(nd2, (64, 1)),
            "w12": np.ascontiguousarray(w12),
            "dyidx": dyidx,
            "posq": posq,
            "wq": np.asarray(inputs["W_q"], np.float32),
            "wk": np.asarray(inputs["W_k"], np.float32),
            "wv": np.asarray(inputs["W_v"], np.float32),
            "wo": np.asarray(inputs["W_o"], np.float32),
            "f1w": np.asarray(inputs["ffn1_w"], np.float32),
            "f2w": np.asarray(inputs["ffn2_w"], np.float32),
            "padmul": padmul, "padadd": padadd, "lcol": lcol,
            "outwT": np.ascontiguousarray(out_w[lo:lo + NSH].T),
            "fpos": np.ascontiguousarray(fpos),
            "pad0": pad0,
        }
        if meta["use_b2"]:
            im["b2col"] = np.asarray(inputs["gcn2_b"], np.float32).reshape(64, 1)
        if meta["use_lng"]:
            im["lng"] = np.tile(np.asarray(inputs["ln_g"], np.float32), (128, 1))
        if meta["use_lnb"]:
            im["lnb"] = np.tile(np.asarray(inputs["ln_b"], np.float32), (128, 1))
        if meta["use_f1b"]:
            im["f1b"] = np.tile(np.asarray(inputs["ffn1_b"], np.float32), (128, 1))
        if meta["use_f2b"]:
            im["f2b"] = np.tile(np.asarray(inputs["ffn2_b"], np.float32), (128, 1))
        if meta["use_ob"]:
            im["obrep"] = np.tile(
                np.asarray(inputs["out_b"], np.float32)[lo:lo + NSH], (128, 1))
        in_maps.append(im)
    return meta, in_maps


# ----------------------------------------------------------------------
# device program
# ----------------------------------------------------------------------

def _build(meta, reps=1):
    NSB1, NSB2 = meta["NSB1"], meta["NSB2"]
    E1, M1 = NSB1 * BIN, NSB1 * 128
    E2, P2 = NSB2 * BIN, NSB2 * 128
    sq = meta["scale_q"]

    nc = bacc.Bacc("TRN2", target_bir_lowering=False, debug=False,
                   enable_asserts=True, num_devices=NCORES)

    def din(name, shape, dt=f32):
        return nc.dram_tensor(name, shape, dt, kind="ExternalInput")

    emb = din("emb", [N, D])
    g1 = din("l1_gidx", [128, E1 // 16], i16)
    d1 = din("l1_dl", [128, E1 // 128])
    c1 = din("l1_cf", [128, E1 // 128])
    nr1 = din("nrep1", [64, M1])
    g2 = din("l2_gidx", [128, E2 // 16], i16)
    d2 = din("l2_dl", [128, E2 // 128])
    c2 = din("l2_cf", [128, E2 // 128])
    nr2 = din("nrep2", [64, P2])
    w12 = din("w12", [64, 64])
    dyidx = din("dyidx", [128, NBC * 16], i16)
    posq = din("posq", [256, 8])
    wq, wk, wv = din("wq", [72, 512]), din("wk", [72, 512]), din("wv", [72, 512])
    wo = din("wo", [512, 72])
    f1w, f2w = din("f1w", [72, 72]), din("f2w", [72, 72])
    padmul_d = din("padmul", [128, 2 * NBC])
    padadd_d = din("padadd", [128, 2 * NBC])
    lcol_d = din("lcol", [128, 2])
    outwT = din("outwT", [72, NSH])
    fpos = din("fpos", [B, NSH])
    pad0 = din("pad0", [128, 2])
    b2col = din("b2col", [64, 1]) if meta["use_b2"] else None
    lng = din("lng", [128, 72]) if meta["use_lng"] else None
    lnb = din("lnb", [128, 72]) if meta["use_lnb"] else None
    f1b = din("f1b", [128, 72]) if meta["use_f1b"] else None
    f2b = din("f2b", [128, 72]) if meta["use_f2b"] else None
    obrep = din("obrep", [128, NSH]) if meta["use_ob"] else None

    phases = os.environ.get("KPHASES", "gcn,tf,lg").split(",")
    small = os.environ.get("KSMALL") == "1"
    assert not (small and "lg" in phases)
    out = nc.dram_tensor("out", [128, 64] if small else [B * L, NSH], f32,
                         kind="ExternalOutput")

    with tile.TileContext(nc) as tc:
      for _rep in range(reps):
       with ExitStack() as es:
        cst = es.enter_context(tc.tile_pool(name="cst", bufs=1))
        glob = es.enter_context(tc.tile_pool(name="glob", bufs=1))
        dramp = es.enter_context(tc.tile_pool(name="dramp", bufs=1,
                                              space="DRAM"))

        # ---------- constants ----------
        iota_i = cst.tile([128, NBLK, 128], i32)
        nc.gpsimd.iota(iota_i[:], [[0, NBLK], [1, 128]], base=0,
                       channel_multiplier=0)
        iota_f = cst.tile([128, NBLK, 128], f32)
        nc.vector.tensor_copy(iota_f[:], iota_i[:])
        ident = cst.tile([128, 128], f32)
        make_identity(nc, ident[:])
        zer200 = cst.tile([128, 200], f32)
        nc.gpsimd.memset(zer200[:], 0.0)
        causal = cst.tile([128, 2, 200], f32)
        for qt in range(2):
            nc.gpsimd.affine_select(
                out=causal[:, qt, :], in_=zer200[:], pattern=[[-1, 200]],
                compare_op=mybir.AluOpType.is_ge, fill=NEG,
                base=128 * qt, channel_multiplier=1)

        lcol_t = cst.tile([128, 2], f32)
        nc.sync.dma_start(lcol_t[:], lcol_d[:, :])
        pad0_t = cst.tile([128, 2], f32)
        nc.sync.dma_start(pad0_t[:], pad0[:, :])
        padmul_t = cst.tile([128, 2 * NBC], f32)
        nc.sync.dma_start(padmul_t[:], padmul_d[:, :])
        padadd_t = cst.tile([128, 2 * NBC], f32)
        nc.sync.dma_start(padadd_t[:], padadd_d[:, :])

        wq_t = glob.tile([72, 512], f32)
        nc.sync.dma_start(wq_t[:], wq[:, :])
        wk_t = glob.tile([72, 512], f32)
        nc.sync.dma_start(wk_t[:], wk[:, :])
        wv_t = glob.tile([72, 512], f32)
        nc.sync.dma_start(wv_t[:], wv[:, :])
        wo_t = glob.tile([128, 4, 72], f32)
        nc.sync.dma_start(wo_t[:], wo.ap().rearrange("(k p) d -> p k d", p=128))
        f1w_t = glob.tile([72, 72], f32)
        nc.sync.dma_start(f1w_t[:], f1w[:, :])
        f2w_t = glob.tile([72, 72], f32)
        nc.sync.dma_start(f2w_t[:], f2w[:, :])
        w12_t = glob.tile([64, 64], f32)
        nc.sync.dma_start(w12_t[:], w12[:, :])
        opt = {}
        for nm, dd, shp in (("lng", lng, [128, 72]), ("lnb", lnb, [128, 72]),
                            ("f1b", f1b, [128, 72]), ("f2b", f2b, [128, 72]),
                            ("b2col", b2col, [64, 1])):
            if dd is not None:
                tt = glob.tile(shp, f32)
                nc.sync.dma_start(tt[:], dd[:, :])
                opt[nm] = tt

        x2loc = dramp.tile([72, NBC * L], f32)
        x2g = dramp.tile([NCORES * 72, NBC * L], f32)
        m_dram = dramp.tile([M1, 64], f32)
        m_glob = dramp.tile([NCORES * M1, 64], f32)
        dyn_dram = dramp.tile([P2, 64], f32)
        x2full = glob.tile([72, B * L], f32)

        # ================= GCN =================
        if "gcn" in phases:
         with (
            tc.tile_pool(name="gcn", bufs=1) as gcn,
            tc.tile_pool(name="msgp", bufs=2) as msgp,
            tc.tile_pool(name="ohp", bufs=2) as ohp,
            tc.tile_pool(name="gps", bufs=3, space="PSUM") as gps,
            tc.tile_pool(name="mps", bufs=3, space="PSUM") as mps,
         ):
            s1T = gcn.tile([64, M1], f32)

            def seg_sum(gsrc_ap, gidx_d, dl_d, cf_d, nsb, outT, nrep_t, extra,
                        tagp):
                gi = gcn.tile([128, (nsb * BIN) // 16], i16, tag="gi" + tagp)
                nc.sync.dma_start(gi[:], gidx_d[:, :])
                dlt = gcn.tile([128, nsb * NBLK], f32, tag="dl" + tagp)
                nc.sync.dma_start(dlt[:], dl_d[:, :])
                cft = gcn.tile([128, nsb * NBLK], f32, tag="cf" + tagp)
                nc.sync.dma_start(cft[:], cf_d[:, :])
                nch = (nsb * BIN) // CHUNK
                bpc = CHUNK // BIN
                for ch in range(nch):
                    msg = msgp.tile([128, CHUNK // 128, 64], f32, tag="msg")
                    nc.gpsimd.dma_gather(
                        msg[:], gsrc_ap,
                        gi[:, ch * (CHUNK // 16):(ch + 1) * (CHUNK // 16)],
                        CHUNK, CHUNK, 64, single_packet=False)
                    for sb in range(bpc):
                        g = ch * bpc + sb
                        oh = ohp.tile([128, NBLK, 128], f32, tag="oh")
                        bsl = slice(g * NBLK, (g + 1) * NBLK)
                        nc.vector.tensor_tensor(
                            out=oh[:], in0=iota_f[:],
                            in1=dlt[:, bsl].to_broadcast([128, NBLK, 128]),
                            op=mybir.AluOpType.is_equal)
                        nc.vector.tensor_tensor(
                            out=oh[:], in0=oh[:],
                            in1=cft[:, bsl].to_broadcast([128, NBLK, 128]),
                            op=mybir.AluOpType.mult)
                        acc = gps.tile([64, 128], f32, space="PSUM", tag="acc")
                        for k in range(NBLK):
                            nc.tensor.matmul(acc[:, :],
                                             msg[:, sb * NBLK + k, :],
                                             oh[:, k, :],
                                             start=(k == 0),
                                             stop=(k == NBLK - 1))
                        sl = outT[:, g * 128:(g + 1) * 128]
                        nc.vector.tensor_tensor(
                            out=sl, in0=acc[:, :],
                            in1=nrep_t[:, g * 128:(g + 1) * 128],
                            op=mybir.AluOpType.mult)
                        if extra is not None:
                            extra(sl)

            nr1_t = gcn.tile([64, M1], f32)
            nc.sync.dma_start(nr1_t[:], nr1[:, :])
            seg_sum(emb[:, :], g1, d1, c1, NSB1, s1T, nr1_t, None, "1")

            # m = s1 @ w12, row-major to DRAM (batched copies + one DMA)
            m_sb = gcn.tile([128, M1 // 128, 64], f32)
            for j in range(M1 // 128):
                pm = mps.tile([128, 64], f32, space="PSUM", tag="pm")
                nc.tensor.matmul(pm[:, :], s1T[:, j * 128:(j + 1) * 128],
                                 w12_t[:], start=True, stop=True)
                nc.scalar.copy(m_sb[:, j, :], pm[:, :])
            nc.sync.dma_start(m_dram[:].rearrange("(c p) d -> p c d", p=128),
                              m_sb[:])
            nc.gpsimd.collective_compute(
                "AllGather", mybir.AluOpType.bypass,
                replica_groups=[list(range(NCORES))],
                ins=[m_dram[:].opt()], outs=[m_glob[:].opt()])

            # L2: m -> dynT
            dynT = gcn.tile([64, P2], f32)

            def add_b2(sl):
                if "b2col" in opt:
                    nc.vector.tensor_scalar(sl, sl, opt["b2col"][:, 0:1], None,
                                            op0=mybir.AluOpType.add)

            nr2_t = gcn.tile([64, P2], f32)
            nc.sync.dma_start(nr2_t[:], nr2[:, :])
            seg_sum(m_glob[:, :], g2, d2, c2, NSB2, dynT, nr2_t,
                    add_b2 if meta["use_b2"] else None, "2")

            # dyn rows to DRAM
            dyn_sb = gcn.tile([128, P2 // 128, 64], f32)
            for j in range(P2 // 128):
                pt = mps.tile([128, 64], f32, space="PSUM", tag="pm")
                nc.tensor.transpose(pt[:, 0:64], dynT[:, j * 128:(j + 1) * 128],
                                    ident[0:64, 0:64])
                nc.scalar.copy(dyn_sb[:, j, :], pt[:, 0:64])
            nc.sync.dma_start(dyn_dram[:].rearrange("(c p) d -> p c d", p=128),
                              dyn_sb[:])

        # ================= transformer (2 batches) =================
        if "tf" in phases:
         with (
            tc.tile_pool(name="tf", bufs=1) as tf,
            tc.tile_pool(name="tfw", bufs=2) as tfw,
            tc.tile_pool(name="psa", bufs=3, space="PSUM") as psa,
            tc.tile_pool(name="psb", bufs=3, space="PSUM") as psb,
         ):
            dyidx_t = tf.tile([128, NBC * 16], i16)
            nc.sync.dma_start(dyidx_t[:], dyidx[:, :])
            for bi in range(NBC):
                x_row = tfw.tile([128, 2, 72], f32, tag="x_row")
                nc.vector.memset(x_row[:], 0.0)
                dy = tfw.tile([128, 2, 64], f32, tag="dy")
                nc.gpsimd.memset(dy[:], 0.0)
                nc.gpsimd.dma_gather(dy[:], dyn_dram[:, :],
                                     dyidx_t[:, bi * 16:(bi + 1) * 16],
                                     256, 200, 64, single_packet=False)
                nc.vector.tensor_copy(x_row[:, :, 0:64], dy[:])
                nc.sync.dma_start(x_row[:, :, 64:72],
                                  posq.ap().rearrange("(c p) d -> p c d",
                                                      p=128))

                def transpose72(src_row, dst_T):
                    for cblk in range(2):
                        pt = psb.tile([128, 128], f32, space="PSUM", tag="ptr")
                        nc.tensor.transpose(pt[0:72, :], src_row[:, cblk, :],
                                            ident[:])
                        nc.scalar.copy(dst_T[:, cblk * 128:(cblk + 1) * 128],
                                       pt[0:72, :])

                x_T = tfw.tile([72, 256], f32, tag="x_T")
                transpose72(x_row, x_T)

                # QKV
                q_T = tfw.tile([128, 4, 200], f32, tag="q_T")
                k_T = tfw.tile([128, 4, 200], f32, tag="k_T")
                for j in range(4):
                    pq = psa.tile([128, 200], f32, space="PSUM", tag="ps")
                    nc.tensor.matmul(pq[:, :], wq_t[:, j * 128:(j + 1) * 128],
                                     x_T[:, 0:200], start=True, stop=True)
                    nc.scalar.mul(q_T[:, j, :], pq[:, :], sq)
                    pk = psa.tile([128, 200], f32, space="PSUM", tag="ps")
                    nc.tensor.matmul(pk[:, :], wk_t[:, j * 128:(j + 1) * 128],
                                     x_T[:, 0:200], start=True, stop=True)
                    nc.scalar.copy(k_T[:, j, :], pk[:, :])
                v_row = tfw.tile([128, 2, 512], f32, tag="v_row")
                pv = psa.tile([128, 512], f32, space="PSUM", tag="ps")
                nc.tensor.matmul(pv[:, :], x_T[:, 0:128], wv_t[:],
                                 start=True, stop=True)
                nc.scalar.copy(v_row[:, 0, :], pv[:, :])
                pv2 = psa.tile([128, 512], f32, space="PSUM", tag="ps")
                nc.tensor.matmul(pv2[0:72, :], x_T[:, 128:200], wv_t[:],
                                 start=True, stop=True)
                nc.scalar.copy(v_row[0:72, 1, :], pv2[0:72, :])

                # attention
                at_T = tfw.tile([128, 2, 8, 200], f32, tag="at_T")
                vat_T = tfw.tile([128, 4, 200], f32, tag="vat_T")
                for qt in range(2):
                    qr = QT_ROWS[qt]
                    qsl = slice(qt * 128, qt * 128 + qr)
                    comb = tfw.tile([128, 200], f32, tag="comb")
                    nc.vector.tensor_scalar(
                        comb[:], causal[:, qt, :],
                        padadd_t[:, 2 * bi + qt:2 * bi + qt + 1], None,
                        op0=mybir.AluOpType.min)
                    sc = tfw.tile([128, 8, 200], f32, tag="sc")
                    nc.gpsimd.memset(sc[:], 0.0)
                    for h in range(8):
                        hr = slice((h % 2) * 64, (h % 2) * 64 + 64)
                        ps_sc = psa.tile([128, 200], f32, space="PSUM",
                                         tag="ps")
                        nc.tensor.matmul(ps_sc[0:qr, :], q_T[hr, h // 2, qsl],
                                         k_T[hr, h // 2, 0:200],
                                         start=True, stop=True)
                        nc.vector.scalar_tensor_tensor(
                            out=sc[0:qr, h, :], in0=ps_sc[0:qr, :],
                            scalar=padmul_t[0:qr, 2 * bi + qt:2 * bi + qt + 1],
                            in1=comb[0:qr, :],
                            op0=mybir.AluOpType.mult, op1=mybir.AluOpType.add)
                    nmax = tfw.tile([128, 8], f32, tag="nmax")
                    nc.vector.tensor_reduce(nmax[0:qr, :], sc[0:qr, :, :],
                                            axis=mybir.AxisListType.X,
                                            op=mybir.AluOpType.max, negate=True)
                    rs = tfw.tile([128, 8], f32, tag="rs")
                    for h in range(8):
                        nc.scalar.activation(
                            sc[0:qr, h, :], sc[0:qr, h, :],
                            mybir.ActivationFunctionType.Exp,
                            bias=nmax[0:qr, h:h + 1], scale=1.0,
                            accum_out=rs[0:qr, h:h + 1])
                    rr = tfw.tile([128, 8], f32, tag="rr")
                    nc.vector.reciprocal(rr[0:qr, :], rs[0:qr, :])
                    for h in range(8):
                        nc.vector.tensor_scalar_mul(sc[0:qr, h, :],
                                                    sc[0:qr, h, :],
                                                    rr[0:qr, h:h + 1])
                    for h in range(8):
                        for kt in range(2):
                            ks = QT_ROWS[kt]
                            pt = psb.tile([128, 128], f32, space="PSUM",
                                          tag="ptr")
                            nc.tensor.transpose(
                                pt[0:ks, 0:qr],
                                sc[0:qr, h, kt * 128:kt * 128 + ks],
                                ident[0:qr, 0:qr])
                            nc.scalar.copy(at_T[0:ks, kt, h, qsl],
                                           pt[0:ks, 0:qr])
                for h in range(8):
                    hr = slice((h % 2) * 64, (h % 2) * 64 + 64)
                    for qt in range(2):
                        qr = QT_ROWS[qt]
                        qsl = slice(qt * 128, qt * 128 + qr)
                        pvt = psa.tile([64, 128], f32, space="PSUM", tag="ps")
                        for kt in range(2):
                            ks = QT_ROWS[kt]
                            nc.tensor.matmul(
                                pvt[:, 0:qr],
                                v_row[0:ks, kt, h * 64:(h + 1) * 64],
                                at_T[0:ks, kt, h, qsl],
                                start=(kt == 0), stop=(kt == 1))
                        nc.scalar.copy(vat_T[hr, h // 2, qsl], pvt[:, 0:qr])

                # att_out + x -> LN -> x1
                y = tfw.tile([128, 2, 72], f32, tag="y")
                nc.gpsimd.memset(y[:], 0.0)
                for qt in range(2):
                    qr = QT_ROWS[qt]
                    qsl = slice(qt * 128, qt * 128 + qr)
                    pat = psa.tile([128, 72], f32, space="PSUM", tag="ps")
                    for kb in range(4):
                        nc.tensor.matmul(pat[0:qr, :], vat_T[:, kb, qsl],
                                         wo_t[:, kb, :],
                                         start=(kb == 0), stop=(kb == 3))
                    nc.vector.tensor_tensor(out=y[0:qr, qt, :],
                                            in0=x_row[0:qr, qt, :],
                                            in1=pat[0:qr, :],
                                            op=mybir.AluOpType.add)

                def layer_norm(y_t, x1_t):
                    sums = tfw.tile([128, 2], f32, tag="lnsum")
                    nc.vector.tensor_reduce(sums[:], y_t[:, :, :],
                                            axis=mybir.AxisListType.X,
                                            op=mybir.AluOpType.add)
                    mu = tfw.tile([128, 2], f32, tag="lnmu")
                    nc.vector.tensor_scalar_mul(mu[:], sums[:], 1.0 / 72.0)
                    vs = tfw.tile([128, 2], f32, tag="lnvs")
                    scr = tfw.tile([128, 2, 72], f32, tag="lnscr")
                    for qt in range(2):
                        nc.vector.tensor_scalar(
                            x1_t[:, qt, :], y_t[:, qt, :], mu[:, qt:qt + 1],
                            None, op0=mybir.AluOpType.subtract)
                        nc.scalar.activation(
                            scr[:, qt, :], x1_t[:, qt, :],
                            mybir.ActivationFunctionType.Square,
                            accum_out=vs[:, qt:qt + 1])
                    sd = tfw.tile([128, 2], f32, tag="lnsd")
                    nc.vector.tensor_scalar(sd[:], vs[:], 1.0 / 72.0, 1e-5,
                                            op0=mybir.AluOpType.mult,
                                            op1=mybir.AluOpType.add)
                    nc.scalar.activation(sd[:], sd[:],
                                         mybir.ActivationFunctionType.Sqrt)
                    rstd = tfw.tile([128, 2], f32, tag="lnrstd")
                    nc.vector.reciprocal(rstd[:], sd[:])
                    for qt in range(2):
                        nc.vector.tensor_scalar_mul(x1_t[:, qt, :],
                                                    x1_t[:, qt, :],
                                                    rstd[:, qt:qt + 1])
                        if "lng" in opt:
                            nc.vector.tensor_tensor(
                                out=x1_t[:, qt, :], in0=x1_t[:, qt, :],
                                in1=opt["lng"][:, :], op=mybir.AluOpType.mult)
                        if "lnb" in opt:
                            nc.vector.tensor_tensor(
                                out=x1_t[:, qt, :], in0=x1_t[:, qt, :],
                                in1=opt["lnb"][:, :], op=mybir.AluOpType.add)

                x1_row = tfw.tile([128, 2, 72], f32, tag="x1_row")
                layer_norm(y, x1_row)

                x1_T = tfw.tile([72, 256], f32, tag="x1_T")
                transpose72(x1_row, x1_T)
                f1_row = tfw.tile([128, 2, 72], f32, tag="f1_row")
                nc.gpsimd.memset(f1_row[:], 0.0)
                for qt in range(2):
                    qr = QT_ROWS[qt]
                    qsl = slice(qt * 128, qt * 128 + qr)
                    pf = psa.tile([128, 72], f32, space="PSUM", tag="ps")
                    nc.tensor.matmul(pf[0:qr, :], x1_T[:, qsl], f1w_t[:],
                                     start=True, stop=True)
                    if "f1b" in opt:
                        nc.vector.tensor_tensor(out=pf[0:qr, :],
                                                in0=pf[0:qr, :],
                                                in1=opt["f1b"][0:qr, :],
                                                op=mybir.AluOpType.add)
                    nc.scalar.activation(f1_row[0:qr, qt, :], pf[0:qr, :],
                                         mybir.ActivationFunctionType.Relu)
                f1_T = tfw.tile([72, 256], f32, tag="f1_T")
                transpose72(f1_row, f1_T)
                y2 = tfw.tile([128, 2, 72], f32, tag="y2")
                nc.gpsimd.memset(y2[:], 0.0)
                for qt in range(2):
                    qr = QT_ROWS[qt]
                    qsl = slice(qt * 128, qt * 128 + qr)
                    pf2 = psa.tile([128, 72], f32, space="PSUM", tag="ps")
                    nc.tensor.matmul(pf2[0:qr, :], f1_T[:, qsl], f2w_t[:],
                                     start=True, stop=True)
                    if "f2b" in opt:
                        nc.vector.tensor_tensor(out=pf2[0:qr, :],
                                                in0=pf2[0:qr, :],
                                                in1=opt["f2b"][0:qr, :],
                                                op=mybir.AluOpType.add)
                    nc.vector.tensor_tensor(out=y2[0:qr, qt, :],
                                            in0=x1_row[0:qr, qt, :],
                                            in1=pf2[0:qr, :],
                                            op=mybir.AluOpType.add)
                x2_row = tfw.tile([128, 2, 72], f32, tag="x2_row")
                layer_norm(y2, x2_row)
                x2_T = tfw.tile([72, 256], f32, tag="x2_T")
                transpose72(x2_row, x2_T)
                nc.sync.dma_start(x2loc[:, bi * L:(bi + 1) * L], x2_T[:, 0:200])

            nc.gpsimd.collective_compute(
                "AllGather", mybir.AluOpType.bypass,
                replica_groups=[list(range(NCORES))],
                ins=[x2loc[:].opt()], outs=[x2g[:].opt()])
            for cc in range(NCORES):
                nc.sync.dma_start(
                    x2full[:, cc * NBC * L:(cc + 1) * NBC * L],
                    x2g[cc * 72:(cc + 1) * 72, :])

        # ================= logits + previous-user mask =================
        if "lg" in phases:
         with (
            tc.tile_pool(name="lg", bufs=2) as lg,
            tc.tile_pool(name="lgc", bufs=1) as lgc,
            tc.tile_pool(name="pslg", bufs=6, space="PSUM") as pslg,
         ):
            outwT_t = lgc.tile([72, NSH], f32)
            nc.sync.dma_start(outwT_t[:], outwT[:, :])
            negf = lgc.tile([128, NSH], f32)
            nc.gpsimd.memset(negf[:], float("-inf"))
            ob_t = None
            if obrep is not None:
                ob_t = lgc.tile([128, NSH], f32)
                nc.sync.dma_start(ob_t[:], obrep[:, :])
            for b in range(B):
                fp_rep = lg.tile([128, NSH], f32, tag="fp")
                nc.sync.dma_start(fp_rep[:],
                                  fpos[b:b + 1, :].to_broadcast([128, NSH]))
                for qt in range(2):
                    qr = QT_ROWS[qt]
                    msk = lg.tile([128, NSH], i8, tag="msk")
                    nc.vector.tensor_scalar(
                        msk[0:qr, :], fp_rep[0:qr, :],
                        lcol_t[0:qr, qt:qt + 1], None,
                        op0=mybir.AluOpType.is_le)
                    ot = lg.tile([128, NSH], f32, tag="ot")
                    for (v0, w) in VT:
                        plg = pslg.tile([128, 512], f32, space="PSUM",
                                        tag="plg")
                        nc.tensor.matmul(
                            plg[0:qr, 0:w],
                            x2full[:, b * L + qt * 128: b * L + qt * 128 + qr],
                            outwT_t[:, v0:v0 + w], start=True, stop=True)
                        if ob_t is not None:
                            nc.vector.tensor_tensor(
                                out=ot[0:qr, v0:v0 + w], in0=plg[0:qr, 0:w],
                                in1=ob_t[0:qr, v0:v0 + w],
                                op=mybir.AluOpType.add)
                        else:
                            nc.scalar.copy(ot[0:qr, v0:v0 + w],
                                           plg[0:qr, 0:w])
                    nc.vector.copy_predicated(ot[0:qr, :], msk[0:qr, :],
                                              negf[0:qr, :])
                    nc.vector.tensor_tensor(
                        out=ot[0:qr, 0:1], in0=ot[0:qr, 0:1],
                        in1=pad0_t[0:qr, qt:qt + 1], op=mybir.AluOpType.add)
                    r0 = b * L + qt * 128
                    nc.sync.dma_start(out[r0:r0 + qr, :], ot[0:qr, :])

    nc.compile()
    return nc


# ----------------------------------------------------------------------
# entry point
# ----------------------------------------------------------------------

def kernel(**inputs):
    meta, in_maps = _host_prep(inputs)
    key = tuple(sorted(meta.items()))
    if key not in _CACHE:
        _CACHE[key] = _build(meta)
    nc = _CACHE[key]
    res = bass_utils.run_bass_kernel_spmd(nc, in_maps,
                                          core_ids=list(range(NCORES)))
    return np.concatenate([res.results[c]["out"] for c in range(NCORES)],
                          axis=1)


# revision 21
# speedup vs baseline: 210.8837x; 179.5180x over previous
"""DyHGCN_H forward as a Trainium2 Bass kernel, SPMD over 8 NeuronCores.

Sharding:
  - GCN + transformer: data-parallel over batch (2 sequences per core). The
    GCN is evaluated sparsely: only the (time-slice, node) pairs actually
    gathered by dyemb are computed, via their 2-hop in-neighborhoods.
  - Aggregation is a gather + one-hot matmul segment-sum (edges sorted by
    destination, packed in 2048-edge bins, accumulated in PSUM).
  - x2 is all-gathered (72 x 3200), then the [30000, 72] output projection +
    previous-user mask run tensor-parallel over the vocab axis (3750/core).
"""
import os
import sys
from contextlib import ExitStack

for _p in ("/opt/trn_rl_repo", "/root/.axon_site/_ro/trn_rl_repo"):
    if os.path.isdir(_p) and _p not in sys.path:
        sys.path.insert(0, _p)

import numpy as np
from concourse import bacc, mybir, tile
from concourse import bass_utils
from concourse.masks import make_identity

f32 = mybir.dt.float32
i16 = mybir.dt.int16
i32 = mybir.dt.int32
i8 = mybir.dt.int8

B, L1TOT, N, D, T = 16, 201, 30000, 64, 8
L, INP, H, DK = 200, 72, 8, 64
STEP = 5
NCORES, NBC = 8, 2          # cores, batches per core
NSH = N // NCORES           # vocab shard: 3750
NEG = float(-2 ** 32 + 1)
BIN = 2048                  # edges per superblock (one 128-wide PSUM window)
NBLK = BIN // 128           # matmul blocks per bin
CHUNK = 8192                # edges per dma_gather call
QT_ROWS = (128, 72)         # rows per q-tile (200 = 128 + 72)
VT = [(v, min(512, NSH - v)) for v in range(0, NSH, 512)]  # vocab tiles

_CACHE = {}


# ----------------------------------------------------------------------
# host-side index packing
# ----------------------------------------------------------------------

def _wrap16(idx, cap):
    """idx j -> [16, cap/16] at (j%16, j//16), tiled to 128 partitions."""
    buf = np.full(cap, -1, np.int16)
    buf[: len(idx)] = np.asarray(idx, np.int16)
    return np.tile(buf.reshape(cap // 16, 16).T, (8, 1))


def _wrap128(vals, cap, fill):
    v = np.full(cap, fill, np.float32)
    v[: len(vals)] = vals
    return np.ascontiguousarray(v.reshape(cap // 128, 128).T)


class _Packer:
    """Pack per-destination edge groups into BIN-edge superblocks, each
    covering <=128 distinct destinations; assigns padded local ids."""

    def __init__(self):
        self.src = []
        self.dl = []
        self.cf = []
        self.norm_d = []       # per padded id (len nbins*128)
        self.nbins = 0
        self._used_e = 0
        self._used_d = 0

    def _flush(self):
        if self._used_e or self._used_d:
            pad = BIN - self._used_e
            if pad:
                self.src.append(np.zeros(pad, np.int64))
                self.dl.append(np.full(pad, -1.0, np.float32))
                self.cf.append(np.zeros(pad, np.float32))
            self.norm_d.extend([0.0] * (128 - self._used_d))
            self.nbins += 1
            self._used_e = 0
            self._used_d = 0

    def add(self, srcs, cfs, norm_dst):
        k = len(srcs)
        assert 0 < k <= BIN, k
        if self._used_e + k > BIN or self._used_d >= 128:
            self._flush()
        pid = self.nbins * 128 + self._used_d
        self.src.append(np.asarray(srcs, np.int64))
        self.cf.append(np.asarray(cfs, np.float32))
        self.dl.append(np.full(k, float(self._used_d), np.float32))
        self.norm_d.append(float(norm_dst))
        self._used_e += k
        self._used_d += 1
        return pid

    def finish(self, nbins_cap):
        self._flush()
        assert self.nbins <= nbins_cap, (self.nbins, nbins_cap)
        e_cap = nbins_cap * BIN
        src = np.concatenate(self.src) if self.src else np.zeros(0, np.int64)
        dl = np.concatenate(self.dl) if self.dl else np.zeros(0, np.float32)
        cf = np.concatenate(self.cf) if self.cf else np.zeros(0, np.float32)
        nd = np.zeros(nbins_cap * 128, np.float32)
        nd[: len(self.norm_d)] = self.norm_d
        src_full = np.zeros(e_cap, np.int64)   # pad with valid idx 0
        src_full[: len(src)] = src
        return (_wrap16(src_full, e_cap), _wrap128(dl, e_cap, -1.0),
                _wrap128(cf, e_cap, 0.0), nd)


def _host_prep(inputs):
    inp = np.asarray(inputs["input"])[:, :-1].astype(np.int64)
    ts = np.asarray(inputs["input_timestamp"])[:, :-1].astype(np.int64)
    ei = np.asarray(inputs["edge_index"])

    blk_max = ts.reshape(B, L // STEP, STEP).max(axis=(0, 2))
    his_pos = np.repeat(np.clip(blk_max - 1, 0, T - 1), STEP)  # [200]

    active = sorted(set(int(t) for t in his_pos))
    sl = {}
    for t in active:
        dst = ei[t, 1].astype(np.int64)
        src = ei[t, 0].astype(np.int64)
        order = np.argsort(dst, kind="stable")
        ds, ss = dst[order], src[order]
        starts = np.searchsorted(ds, np.arange(N))
        ends = np.searchsorted(ds, np.arange(N), side="right")
        deg = (np.bincount(dst, minlength=N) + 1.0).astype(np.float32)
        norm = (1.0 / np.sqrt(deg)).astype(np.float32)
        sl[t] = (ss, starts, ends, norm)

    # ---- global L1 node set (deduped across cores), sharded round-robin ----
    l1_id = {}
    l1_nodes = []

    def _l1(t, n):
        if (t, n) not in l1_id:
            l1_id[(t, n)] = len(l1_nodes)
            l1_nodes.append((t, n))
        return l1_id[(t, n)]

    cores = []
    for c in range(NCORES):
        bsel = range(NBC * c, NBC * (c + 1))
        pair_id = {}
        pairs = []
        pos_pair = np.zeros((NBC, L), np.int64)
        for bi, b in enumerate(bsel):
            for l in range(L):
                key = (int(his_pos[l]), int(inp[b, l]))
                if key not in pair_id:
                    pair_id[key] = len(pairs)
                    pairs.append(key)
                pos_pair[bi, l] = pair_id[key]
        pair_edges = []
        for (t, n) in pairs:
            ss, st, en, norm = sl[t]
            srcs = ss[st[n]:en[n]]
            pair_edges.append((t, n, srcs))
            _l1(t, n)
            for s in srcs:
                _l1(t, int(s))
        cores.append(dict(pairs=pairs, pair_edges=pair_edges,
                          pos_pair=pos_pair))

    # pack each core's shard of the global L1 set
    l1_pid_glob = np.zeros(len(l1_nodes), np.int64)   # -> (core, local pid)
    l1_core = np.zeros(len(l1_nodes), np.int64)
    pk1s = [_Packer() for _ in range(NCORES)]
    for q, (t, n) in enumerate(l1_nodes):
        c = q % NCORES
        ss, st, en, norm = sl[t]
        srcs = np.concatenate([ss[st[n]:en[n]], [n]])          # + self
        l1_pid_glob[q] = pk1s[c].add(srcs, norm[srcs], norm[n])
        l1_core[q] = c

    # ---- pack L2 per core (srcs reference the global gathered m table) ----
    for c in range(NCORES):
        cd = cores[c]
        pk2 = _Packer()
        p2_pid = np.zeros(len(cd["pairs"]), np.int64)
        for p, (t, n, srcs) in enumerate(cd["pair_edges"]):
            ss, st, en, norm = sl[t]
            loc = np.array([l1_id[(t, int(s))] for s in srcs] +
                           [l1_id[(t, n)]], np.int64)
            cfs = np.concatenate([norm[srcs], [norm[n]]])
            p2_pid[p] = pk2.add(loc, cfs, norm[n])   # loc -> global pid later
        cd["pk2"] = pk2
        cd["p2_pid"] = p2_pid

    def _ceil(x, m):
        return ((x + m - 1) // m) * m

    NSB1 = _ceil(max(pk.nbins + 1 for pk in pk1s), CHUNK // BIN)
    NSB2 = _ceil(max(c["pk2"].nbins + 1 for c in cores), CHUNK // BIN)
    M1, P2 = NSB1 * 128, NSB2 * 128
    assert NCORES * M1 <= 32000 and P2 <= 32000
    # global padded id for each dense l1 id
    l1_glob = l1_core * M1 + l1_pid_glob

    w12 = (inputs["gcn1_w"].astype(np.float32) @
           inputs["gcn2_w"].astype(np.float32))
    assert not np.any(inputs["gcn1_b"]), "gcn1_b folding requires zeros"

    pos_emb = np.asarray(inputs["pos_emb"], np.float32)
    posq = np.zeros((256, 8), np.float32)
    posq[:L] = pos_emb[:L]

    lcol = np.full((128, 2), 1e9, np.float32)
    lcol[:, 0] = np.arange(128)
    lcol[:72, 1] = np.arange(128, 200)

    out_w = np.asarray(inputs["out_w"], np.float32)
    scale_q = np.float32(1.0 / (np.sqrt(np.float32(DK)) + 1e-6))

    meta = dict(NSB1=NSB1, NSB2=NSB2,
                use_b2=bool(np.any(inputs["gcn2_b"])),
                use_lng=bool(np.any(inputs["ln_g"] != 1.0)),
                use_lnb=bool(np.any(inputs["ln_b"])),
                use_f1b=bool(np.any(inputs["ffn1_b"])),
                use_f2b=bool(np.any(inputs["ffn2_b"])),
                use_ob=bool(np.any(inputs["out_b"])),
                scale_q=float(scale_q))

    in_maps = []
    for c, cd in enumerate(cores):
        g1, d1, c1, nd1 = pk1s[c].finish(NSB1)
        pk2 = cd["pk2"]
        pk2.src = [l1_glob[s] for s in pk2.src]
        g2, d2, c2, nd2 = pk2.finish(NSB2)

        dyidx = np.zeros((128, NBC * 16), np.int16)
        for bi in range(NBC):
            pid = cd["p2_pid"][cd["pos_pair"][bi]]
            dyidx[:, bi * 16:(bi + 1) * 16] = _wrap16(pid, 256)

        bsel = range(NBC * c, NBC * (c + 1))
        padmul = np.ones((128, 2 * NBC), np.float32)
        padadd = np.zeros((128, 2 * NBC), np.float32)
        for bi, b in enumerate(bsel):
            for qt in range(2):
                rows = QT_ROWS[qt]
                padv = (inp[b, qt * 128:qt * 128 + rows] == 0)
                padmul[:rows, 2 * bi + qt] = np.where(padv, 0.0, 1.0)
                padadd[:rows, 2 * bi + qt] = np.where(padv, NEG, 0.0)

        fpos = np.full((B, NSH), 1e9, np.float32)
        lo = c * NSH
        for b in range(B):
            u = inp[b]
            m = (u >= lo) & (u < lo + NSH)
            np.minimum.at(fpos[b], (u[m] - lo).astype(np.int64),
                          np.arange(L, dtype=np.float32)[m])
        pad0 = np.zeros((128, 2), np.float32)
        if lo == 0:
            # reference's tril-zeros mask col 0 for l<=198 only
            for qt in range(2):
                rows = QT_ROWS[qt]
                lv = qt * 128 + np.arange(rows)
                pad0[:rows, qt] = np.where(lv <= 198, -np.inf, 0.0)

        im = {
            "emb": np.asarray(inputs["emb_g"], np.float32),
            "l1_gidx": g1, "l1_dl": d1, "l1_cf": c1,
            "nrep1": np.tile(nd1, (64, 1)),
            "l2_gidx": g2, "l2_dl": d2, "l2_cf": c2,
            "nrep2": np.tile(nd2, (64, 1)),
            "w12": np.ascontiguousarray(w12),
            "dyidx": dyidx,
            "posq": posq,
            "wq": np.asarray(inputs["W_q"], np.float32),
            "wk": np.asarray(inputs["W_k"], np.float32),
            "wv": np.asarray(inputs["W_v"], np.float32),
            "wo": np.asarray(inputs["W_o"], np.float32),
            "f1w": np.asarray(inputs["ffn1_w"], np.float32),
            "f2w": np.asarray(inputs["ffn2_w"], np.float32),
            "padmul": padmul, "padadd": padadd, "lcol": lcol,
            "outwT": np.ascontiguousarray(out_w[lo:lo + NSH].T),
            "fpos": np.ascontiguousarray(fpos),
            "pad0": pad0,
        }
        if meta["use_b2"]:
            im["b2col"] = np.asarray(inputs["gcn2_b"], np.float32).reshape(64, 1)
        if meta["use_lng"]:
            im["lng"] = np.tile(np.asarray(inputs["ln_g"], np.float32), (128, 1))
        if meta["use_lnb"]:
            im["lnb"] = np.tile(np.asarray(inputs["ln_b"], np.float32), (128, 1))
        if meta["use_f1b"]:
            im["f1b"] = np.tile(np.asarray(inputs["ffn1_b"], np.float32), (128, 1))
        if meta["use_f2b"]:
            im["f2b"] = np.tile(np.asarray(inputs["ffn2_b"], np.float32), (128, 1))
        if meta["use_ob"]:
            im["obrep"] = np.tile(
                np.asarray(inputs["out_b"], np.float32)[lo:lo + NSH], (128, 1))
        in_maps.append(im)
    return meta, in_maps


# ----------------------------------------------------------------------
# device program
# ----------------------------------------------------------------------

def _build(meta, reps=1):
    NSB1, NSB2 = meta["NSB1"], meta["NSB2"]
    E1, M1 = NSB1 * BIN, NSB1 * 128
    E2, P2 = NSB2 * BIN, NSB2 * 128
    sq = meta["scale_q"]

    nc = bacc.Bacc("TRN2", target_bir_lowering=False, debug=False,
                   enable_asserts=True, num_devices=NCORES)

    def din(name, shape, dt=f32):
        return nc.dram_tensor(name, shape, dt, kind="ExternalInput")

    emb = din("emb", [N, D])
    g1 = din("l1_gidx", [128, E1 // 16], i16)
    d1 = din("l1_dl", [128, E1 // 128])
    c1 = din("l1_cf", [128, E1 // 128])
    nr1 = din("nrep1", [64, M1])
    g2 = din("l2_gidx", [128, E2 // 16], i16)
    d2 = din("l2_dl", [128, E2 // 128])
    c2 = din("l2_cf", [128, E2 // 128])
    nr2 = din("nrep2", [64, P2])
    w12 = din("w12", [64, 64])
    dyidx = din("dyidx", [128, NBC * 16], i16)
    posq = din("posq", [256, 8])
    wq, wk, wv = din("wq", [72, 512]), din("wk", [72, 512]), din("wv", [72, 512])
    wo = din("wo", [512, 72])
    f1w, f2w = din("f1w", [72, 72]), din("f2w", [72, 72])
    padmul_d = din("padmul", [128, 2 * NBC])
    padadd_d = din("padadd", [128, 2 * NBC])
    lcol_d = din("lcol", [128, 2])
    outwT = din("outwT", [72, NSH])
    fpos = din("fpos", [B, NSH])
    pad0 = din("pad0", [128, 2])
    b2col = din("b2col", [64, 1]) if meta["use_b2"] else None
    lng = din("lng", [128, 72]) if meta["use_lng"] else None
    lnb = din("lnb", [128, 72]) if meta["use_lnb"] else None
    f1b = din("f1b", [128, 72]) if meta["use_f1b"] else None
    f2b = din("f2b", [128, 72]) if meta["use_f2b"] else None
    obrep = din("obrep", [128, NSH]) if meta["use_ob"] else None

    phases = os.environ.get("KPHASES", "gcn,tf,lg").split(",")
    small = os.environ.get("KSMALL") == "1"
    assert not (small and "lg" in phases)
    out = nc.dram_tensor("out", [128, 64] if small else [B * L, NSH], f32,
                         kind="ExternalOutput")

    with tile.TileContext(nc) as tc:
      for _rep in range(reps):
       with ExitStack() as es:
        cst = es.enter_context(tc.tile_pool(name="cst", bufs=1))
        glob = es.enter_context(tc.tile_pool(name="glob", bufs=1))
        dramp = es.enter_context(tc.tile_pool(name="dramp", bufs=1,
                                              space="DRAM"))

        # ---------- constants ----------
        iota_i = cst.tile([128, NBLK, 128], i32)
        nc.gpsimd.iota(iota_i[:], [[0, NBLK], [1, 128]], base=0,
                       channel_multiplier=0)
        iota_f = cst.tile([128, NBLK, 128], f32)
        nc.vector.tensor_copy(iota_f[:], iota_i[:])
        ident = cst.tile([128, 128], f32)
        make_identity(nc, ident[:])
        zer200 = cst.tile([128, 200], f32)
        nc.gpsimd.memset(zer200[:], 0.0)
        causal = cst.tile([128, 2, 200], f32)
        for qt in range(2):
            nc.gpsimd.affine_select(
                out=causal[:, qt, :], in_=zer200[:], pattern=[[-1, 200]],
                compare_op=mybir.AluOpType.is_ge, fill=NEG,
                base=128 * qt, channel_multiplier=1)

        lcol_t = cst.tile([128, 2], f32)
        nc.sync.dma_start(lcol_t[:], lcol_d[:, :])
        pad0_t = cst.tile([128, 2], f32)
        nc.sync.dma_start(pad0_t[:], pad0[:, :])
        padmul_t = cst.tile([128, 2 * NBC], f32)
        nc.sync.dma_start(padmul_t[:], padmul_d[:, :])
        padadd_t = cst.tile([128, 2 * NBC], f32)
        nc.sync.dma_start(padadd_t[:], padadd_d[:, :])

        wq_t = glob.tile([72, 512], f32)
        nc.sync.dma_start(wq_t[:], wq[:, :])
        wk_t = glob.tile([72, 512], f32)
        nc.sync.dma_start(wk_t[:], wk[:, :])
        wv_t = glob.tile([72, 512], f32)
        nc.sync.dma_start(wv_t[:], wv[:, :])
        wo_t = glob.tile([128, 4, 72], f32)
        nc.sync.dma_start(wo_t[:], wo.ap().rearrange("(k p) d -> p k d", p=128))
        f1w_t = glob.tile([72, 72], f32)
        nc.sync.dma_start(f1w_t[:], f1w[:, :])
        f2w_t = glob.tile([72, 72], f32)
        nc.sync.dma_start(f2w_t[:], f2w[:, :])
        w12_t = glob.tile([64, 64], f32)
        nc.sync.dma_start(w12_t[:], w12[:, :])
        opt = {}
        for nm, dd, shp in (("lng", lng, [128, 72]), ("lnb", lnb, [128, 72]),
                            ("f1b", f1b, [128, 72]), ("f2b", f2b, [128, 72]),
                            ("b2col", b2col, [64, 1])):
            if dd is not None:
                tt = glob.tile(shp, f32)
                nc.sync.dma_start(tt[:], dd[:, :])
                opt[nm] = tt

        x2loc = dramp.tile([72, NBC * L], f32)
        x2g = dramp.tile([NCORES * 72, NBC * L], f32)
        m_dram = dramp.tile([M1, 64], f32)
        m_glob = dramp.tile([NCORES * M1, 64], f32)
        dyn_dram = dramp.tile([P2, 64], f32)
        x2full = glob.tile([72, B * L], f32)

        # ================= GCN =================
        if "gcn" in phases:
         with (
            tc.tile_pool(name="gcn", bufs=1) as gcn,
            tc.tile_pool(name="msgp", bufs=2) as msgp,
            tc.tile_pool(name="ohp", bufs=2) as ohp,
            tc.tile_pool(name="gps", bufs=3, space="PSUM") as gps,
            tc.tile_pool(name="mps", bufs=3, space="PSUM") as mps,
         ):
            s1T = gcn.tile([64, M1], f32)

            def seg_sum(gsrc_ap, gidx_d, dl_d, cf_d, nsb, outT, nrep_t, extra,
                        tagp):
                gi = gcn.tile([128, (nsb * BIN) // 16], i16, tag="gi" + tagp)
                nc.sync.dma_start(gi[:], gidx_d[:, :])
                dlt = gcn.tile([128, nsb * NBLK], f32, tag="dl" + tagp)
                nc.sync.dma_start(dlt[:], dl_d[:, :])
                cft = gcn.tile([128, nsb * NBLK], f32, tag="cf" + tagp)
                nc.sync.dma_start(cft[:], cf_d[:, :])
                nch = (nsb * BIN) // CHUNK
                bpc = CHUNK // BIN
                for ch in range(nch):
                    msg = msgp.tile([128, CHUNK // 128, 64], f32, tag="msg")
                    nc.gpsimd.dma_gather(
                        msg[:], gsrc_ap,
                        gi[:, ch * (CHUNK // 16):(ch + 1) * (CHUNK // 16)],
                        CHUNK, CHUNK, 64, single_packet=False)
                    for sb in range(bpc):
                        g = ch * bpc + sb
                        oh = ohp.tile([128, NBLK, 128], f32, tag="oh")
                        bsl = slice(g * NBLK, (g + 1) * NBLK)
                        nc.vector.tensor_tensor(
                            out=oh[:], in0=iota_f[:],
                            in1=dlt[:, bsl].to_broadcast([128, NBLK, 128]),
                            op=mybir.AluOpType.is_equal)
                        nc.vector.tensor_tensor(
                            out=oh[:], in0=oh[:],
                            in1=cft[:, bsl].to_broadcast([128, NBLK, 128]),
                            op=mybir.AluOpType.mult)
                        acc = gps.tile([64, 128], f32, space="PSUM", tag="acc")
                        for k in range(NBLK):
                            nc.tensor.matmul(acc[:, :],
                                             msg[:, sb * NBLK + k, :],
                                             oh[:, k, :],
                                             start=(k == 0),
                                             stop=(k == NBLK - 1))
                        sl = outT[:, g * 128:(g + 1) * 128]
                        nc.vector.tensor_tensor(
                            out=sl, in0=acc[:, :],
                            in1=nrep_t[:, g * 128:(g + 1) * 128],
                            op=mybir.AluOpType.mult)
                        if extra is not None:
                            extra(sl)

            nr1_t = gcn.tile([64, M1], f32)
            nc.sync.dma_start(nr1_t[:], nr1[:, :])
            seg_sum(emb[:, :], g1, d1, c1, NSB1, s1T, nr1_t, None, "1")

            # m = s1 @ w12, row-major to DRAM (batched copies + one DMA)
            m_sb = gcn.tile([128, M1 // 128, 64], f32)
            for j in range(M1 // 128):
                pm = mps.tile([128, 64], f32, space="PSUM", tag="pm")
                nc.tensor.matmul(pm[:, :], s1T[:, j * 128:(j + 1) * 128],
                                 w12_t[:], start=True, stop=True)
                nc.scalar.copy(m_sb[:, j, :], pm[:, :])
            nc.sync.dma_start(m_dram[:].rearrange("(c p) d -> p c d", p=128),
                              m_sb[:])
            nc.gpsimd.collective_compute(
                "AllGather", mybir.AluOpType.bypass,
                replica_groups=[list(range(NCORES))],
                ins=[m_dram[:].opt()], outs=[m_glob[:].opt()])

            # L2: m -> dynT
            dynT = gcn.tile([64, P2], f32)

            def add_b2(sl):
                if "b2col" in opt:
                    nc.vector.tensor_scalar(sl, sl, opt["b2col"][:, 0:1], None,
                                            op0=mybir.AluOpType.add)

            nr2_t = gcn.tile([64, P2], f32)
            nc.sync.dma_start(nr2_t[:], nr2[:, :])
            seg_sum(m_glob[:, :], g2, d2, c2, NSB2, dynT, nr2_t,
                    add_b2 if meta["use_b2"] else None, "2")

            # dyn rows to DRAM
            dyn_sb = gcn.tile([128, P2 // 128, 64], f32)
            for j in range(P2 // 128):
                pt = mps.tile([128, 64], f32, space="PSUM", tag="pm")
                nc.tensor.transpose(pt[:, 0:64], dynT[:, j * 128:(j + 1) * 128],
                                    ident[0:64, 0:64])
                nc.scalar.copy(dyn_sb[:, j, :], pt[:, 0:64])
            nc.sync.dma_start(dyn_dram[:].rearrange("(c p) d -> p c d", p=128),
                              dyn_sb[:])

        # ================= transformer (2 batches) =================
        if "tf" in phases:
         with (
            tc.tile_pool(name="tf", bufs=1) as tf,
            tc.tile_pool(name="tfw", bufs=2) as tfw,
            tc.tile_pool(name="psa", bufs=3, space="PSUM") as psa,
            tc.tile_pool(name="psb", bufs=3, space="PSUM") as psb,
         ):
            dyidx_t = tf.tile([128, NBC * 16], i16)
            nc.sync.dma_start(dyidx_t[:], dyidx[:, :])
            for bi in range(NBC):
                x_row = tfw.tile([128, 2, 72], f32, tag="x_row")
                nc.vector.memset(x_row[:], 0.0)
                dy = tfw.tile([128, 2, 64], f32, tag="dy")
                nc.gpsimd.memset(dy[:], 0.0)
                nc.gpsimd.dma_gather(dy[:], dyn_dram[:, :],
                                     dyidx_t[:, bi * 16:(bi + 1) * 16],
                                     256, 200, 64, single_packet=False)
                nc.vector.tensor_copy(x_row[:, :, 0:64], dy[:])
                nc.sync.dma_start(x_row[:, :, 64:72],
                                  posq.ap().rearrange("(c p) d -> p c d",
                                                      p=128))

                def transpose72(src_row, dst_T):
                    for cblk in range(2):
                        pt = psb.tile([128, 128], f32, space="PSUM", tag="ptr")
                        nc.tensor.transpose(pt[0:72, :], src_row[:, cblk, :],
                                            ident[:])
                        nc.scalar.copy(dst_T[:, cblk * 128:(cblk + 1) * 128],
                                       pt[0:72, :])

                x_T = tfw.tile([72, 256], f32, tag="x_T")
                transpose72(x_row, x_T)

                # QKV
                q_T = tfw.tile([128, 4, 200], f32, tag="q_T")
                k_T = tfw.tile([128, 4, 200], f32, tag="k_T")
                for j in range(4):
                    pq = psa.tile([128, 200], f32, space="PSUM", tag="ps")
                    nc.tensor.matmul(pq[:, :], wq_t[:, j * 128:(j + 1) * 128],
                                     x_T[:, 0:200], start=True, stop=True)
                    nc.scalar.mul(q_T[:, j, :], pq[:, :], sq)
                    pk = psa.tile([128, 200], f32, space="PSUM", tag="ps")
                    nc.tensor.matmul(pk[:, :], wk_t[:, j * 128:(j + 1) * 128],
                                     x_T[:, 0:200], start=True, stop=True)
                    nc.scalar.copy(k_T[:, j, :], pk[:, :])
                v_row = tfw.tile([128, 2, 512], f32, tag="v_row")
                pv = psa.tile([128, 512], f32, space="PSUM", tag="ps")
                nc.tensor.matmul(pv[:, :], x_T[:, 0:128], wv_t[:],
                                 start=True, stop=True)
                nc.scalar.copy(v_row[:, 0, :], pv[:, :])
                pv2 = psa.tile([128, 512], f32, space="PSUM", tag="ps")
                nc.tensor.matmul(pv2[0:72, :], x_T[:, 128:200], wv_t[:],
                                 start=True, stop=True)
                nc.scalar.copy(v_row[0:72, 1, :], pv2[0:72, :])

                # attention
                at_T = tfw.tile([128, 2, 8, 200], f32, tag="at_T")
                vat_T = tfw.tile([128, 4, 200], f32, tag="vat_T")
                for qt in range(2):
                    qr = QT_ROWS[qt]
                    qsl = slice(qt * 128, qt * 128 + qr)
                    comb = tfw.tile([128, 200], f32, tag="comb")
                    nc.vector.tensor_scalar(
                        comb[:], causal[:, qt, :],
                        padadd_t[:, 2 * bi + qt:2 * bi + qt + 1], None,
                        op0=mybir.AluOpType.min)
                    sc = tfw.tile([128, 8, 200], f32, tag="sc")
                    nc.gpsimd.memset(sc[:], 0.0)
                    for h in range(8):
                        hr = slice((h % 2) * 64, (h % 2) * 64 + 64)
                        ps_sc = psa.tile([128, 200], f32, space="PSUM",
                                         tag="ps")
                        nc.tensor.matmul(ps_sc[0:qr, :], q_T[hr, h // 2, qsl],
                                         k_T[hr, h // 2, 0:200],
                                         start=True, stop=True)
                        nc.vector.scalar_tensor_tensor(
                            out=sc[0:qr, h, :], in0=ps_sc[0:qr, :],
                            scalar=padmul_t[0:qr, 2 * bi + qt:2 * bi + qt + 1],
                            in1=comb[0:qr, :],
                            op0=mybir.AluOpType.mult, op1=mybir.AluOpType.add)
                    nmax = tfw.tile([128, 8], f32, tag="nmax")
                    nc.vector.tensor_reduce(nmax[0:qr, :], sc[0:qr, :, :],
                                            axis=mybir.AxisListType.X,
                                            op=mybir.AluOpType.max, negate=True)
                    rs = tfw.tile([128, 8], f32, tag="rs")
                    for h in range(8):
                        nc.scalar.activation(
                            sc[0:qr, h, :], sc[0:qr, h, :],
                            mybir.ActivationFunctionType.Exp,
                            bias=nmax[0:qr, h:h + 1], scale=1.0,
                            accum_out=rs[0:qr, h:h + 1])
                    rr = tfw.tile([128, 8], f32, tag="rr")
                    nc.vector.reciprocal(rr[0:qr, :], rs[0:qr, :])
                    for h in range(8):
                        nc.vector.tensor_scalar_mul(sc[0:qr, h, :],
                                                    sc[0:qr, h, :],
                                                    rr[0:qr, h:h + 1])
                    for h in range(8):
                        for kt in range(2):
                            ks = QT_ROWS[kt]
                            pt = psb.tile([128, 128], f32, space="PSUM",
                                          tag="ptr")
                            nc.tensor.transpose(
                                pt[0:ks, 0:qr],
                                sc[0:qr, h, kt * 128:kt * 128 + ks],
                                ident[0:qr, 0:qr])
                            nc.scalar.copy(at_T[0:ks, kt, h, qsl],
                                           pt[0:ks, 0:qr])
                for h in range(8):
                    hr = slice((h % 2) * 64, (h % 2) * 64 + 64)
                    for qt in range(2):
                        qr = QT_ROWS[qt]
                        qsl = slice(qt * 128, qt * 128 + qr)
                        pvt = psa.tile([64, 128], f32, space="PSUM", tag="ps")
                        for kt in range(2):
                            ks = QT_ROWS[kt]
                            nc.tensor.matmul(
                                pvt[:, 0:qr],
                                v_row[0:ks, kt, h * 64:(h + 1) * 64],
                                at_T[0:ks, kt, h, qsl],
                                start=(kt == 0), stop=(kt == 1))
                        nc.scalar.copy(vat_T[hr, h // 2, qsl], pvt[:, 0:qr])

                # att_out + x -> LN -> x1
                y = tfw.tile([128, 2, 72], f32, tag="y")
                nc.gpsimd.memset(y[:], 0.0)
                for qt in range(2):
                    qr = QT_ROWS[qt]
                    qsl = slice(qt * 128, qt * 128 + qr)
                    pat = psa.tile([128, 72], f32, space="PSUM", tag="ps")
                    for kb in range(4):
                        nc.tensor.matmul(pat[0:qr, :], vat_T[:, kb, qsl],
                                         wo_t[:, kb, :],
                                         start=(kb == 0), stop=(kb == 3))
                    nc.vector.tensor_tensor(out=y[0:qr, qt, :],
                                            in0=x_row[0:qr, qt, :],
                                            in1=pat[0:qr, :],
                                            op=mybir.AluOpType.add)

                def layer_norm(y_t, x1_t):
                    sums = tfw.tile([128, 2], f32, tag="lnsum")
                    nc.vector.tensor_reduce(sums[:], y_t[:, :, :],
                                            axis=mybir.AxisListType.X,
                                            op=mybir.AluOpType.add)
                    mu = tfw.tile([128, 2], f32, tag="lnmu")
                    nc.vector.tensor_scalar_mul(mu[:], sums[:], 1.0 / 72.0)
                    vs = tfw.tile([128, 2], f32, tag="lnvs")
                    scr = tfw.tile([128, 2, 72], f32, tag="lnscr")
                    for qt in range(2):
                        nc.vector.tensor_scalar(
                            x1_t[:, qt, :], y_t[:, qt, :], mu[:, qt:qt + 1],
                            None, op0=mybir.AluOpType.subtract)
                        nc.scalar.activation(
                            scr[:, qt, :], x1_t[:, qt, :],
                            mybir.ActivationFunctionType.Square,
                            accum_out=vs[:, qt:qt + 1])
                    sd = tfw.tile([128, 2], f32, tag="lnsd")
                    nc.vector.tensor_scalar(sd[:], vs[:], 1.0 / 72.0, 1e-5,
                                            op0=mybir.AluOpType.mult,
                                            op1=mybir.AluOpType.add)
                    nc.scalar.activation(sd[:], sd[:],
                                         mybir.ActivationFunctionType.Sqrt)
                    rstd = tfw.tile([128, 2], f32, tag="lnrstd")
                    nc.vector.reciprocal(rstd[:], sd[:])
                    for qt in range(2):
                        nc.vector.tensor_scalar_mul(x1_t[:, qt, :],
                                                    x1_t[:, qt, :],
                                                    rstd[:, qt:qt + 1])
                        if "lng" in opt:
                            nc.vector.tensor_tensor(
                                out=x1_t[:, qt, :], in0=x1_t[:, qt, :],
                                in1=opt["lng"][:, :], op=mybir.AluOpType.mult)
                        if "lnb" in opt:
                            nc.vector.tensor_tensor(
                                out=x1_t[:, qt, :], in0=x1_t[:, qt, :],
                                in1=opt["lnb"][:, :], op=mybir.AluOpType.add)

                x1_row = tfw.tile([128, 2, 72], f32, tag="x1_row")
                layer_norm(y, x1_row)

                x1_T = tfw.tile([72, 256], f32, tag="x1_T")
                transpose72(x1_row, x1_T)
                f1_row = tfw.tile([128, 2, 72], f32, tag="f1_row")
                nc.gpsimd.memset(f1_row[:], 0.0)
                for qt in range(2):
                    qr = QT_ROWS[qt]
                    qsl = slice(qt * 128, qt * 128 + qr)
                    pf = psa.tile([128, 72], f32, space="PSUM", tag="ps")
                    nc.tensor.matmul(pf[0:qr, :], x1_T[:, qsl], f1w_t[:],
                                     start=True, stop=True)
                    if "f1b" in opt:
                        nc.vector.tensor_tensor(out=pf[0:qr, :],
                                                in0=pf[0:qr, :],
                                                in1=opt["f1b"][0:qr, :],
                                                op=mybir.AluOpType.add)
                    nc.scalar.activation(f1_row[0:qr, qt, :], pf[0:qr, :],
                                         mybir.ActivationFunctionType.Relu)
                f1_T = tfw.tile([72, 256], f32, tag="f1_T")
                transpose72(f1_row, f1_T)
                y2 = tfw.tile([128, 2, 72], f32, tag="y2")
                nc.gpsimd.memset(y2[:], 0.0)
                for qt in range(2):
                    qr = QT_ROWS[qt]
                    qsl = slice(qt * 128, qt * 128 + qr)
                    pf2 = psa.tile([128, 72], f32, space="PSUM", tag="ps")
                    nc.tensor.matmul(pf2[0:qr, :], f1_T[:, qsl], f2w_t[:],
                                     start=True, stop=True)
                    if "f2b" in opt:
                        nc.vector.tensor_tensor(out=pf2[0:qr, :],
                                                in0=pf2[0:qr, :],
                                                in1=opt["f2b"][0:qr, :],
                                                op=mybir.AluOpType.add)
                    nc.vector.tensor_tensor(out=y2[0:qr, qt, :],
                                            in0=x1_row[0:qr, qt, :],
                                            in1=pf2[0:qr, :],
                                            op=mybir.AluOpType.add)
                x2_row = tfw.tile([128, 2, 72], f32, tag="x2_row")
                layer_norm(y2, x2_row)
                x2_T = tfw.tile([72, 256], f32, tag="x2_T")
                transpose72(x2_row, x2_T)
                nc.sync.dma_start(x2loc[:, bi * L:(bi + 1) * L], x2_T[:, 0:200])

            nc.gpsimd.collective_compute(
                "AllGather", mybir.AluOpType.bypass,
                replica_groups=[list(range(NCORES))],
                ins=[x2loc[:].opt()], outs=[x2g[:].opt()])
            for cc in range(NCORES):
                nc.sync.dma_start(
                    x2full[:, cc * NBC * L:(cc + 1) * NBC * L],
                    x2g[cc * 72:(cc + 1) * 72, :])

        # ================= logits + previous-user mask =================
        if "lg" in phases:
         with (
            tc.tile_pool(name="lg", bufs=2) as lg,
            tc.tile_pool(name="lgc", bufs=1) as lgc,
            tc.tile_pool(name="pslg", bufs=6, space="PSUM") as pslg,
         ):
            outwT_t = lgc.tile([72, NSH], f32)
            nc.sync.dma_start(outwT_t[:], outwT[:, :])
            negf = lgc.tile([128, NSH], f32)
            nc.gpsimd.memset(negf[:], float("-inf"))
            ob_t = None
            if obrep is not None:
                ob_t = lgc.tile([128, NSH], f32)
                nc.sync.dma_start(ob_t[:], obrep[:, :])
            for b in range(B):
                fp_rep = lg.tile([128, NSH], f32, tag="fp")
                nc.sync.dma_start(fp_rep[:],
                                  fpos[b:b + 1, :].to_broadcast([128, NSH]))
                for qt in range(2):
                    qr = QT_ROWS[qt]
                    msk = lg.tile([128, NSH], i8, tag="msk")
                    nc.vector.tensor_scalar(
                        msk[0:qr, :], fp_rep[0:qr, :],
                        lcol_t[0:qr, qt:qt + 1], None,
                        op0=mybir.AluOpType.is_le)
                    ot = lg.tile([128, NSH], f32, tag="ot")
                    for (v0, w) in VT:
                        plg = pslg.tile([128, 512], f32, space="PSUM",
                                        tag="plg")
                        nc.tensor.matmul(
                            plg[0:qr, 0:w],
                            x2full[:, b * L + qt * 128: b * L + qt * 128 + qr],
                            outwT_t[:, v0:v0 + w], start=True, stop=True)
                        if ob_t is not None:
                            nc.vector.tensor_tensor(
                                out=ot[0:qr, v0:v0 + w], in0=plg[0:qr, 0:w],
                                in1=ob_t[0:qr, v0:v0 + w],
                                op=mybir.AluOpType.add)
                        else:
                            nc.scalar.copy(ot[0:qr, v0:v0 + w],
                                           plg[0:qr, 0:w])
                    nc.vector.copy_predicated(ot[0:qr, :], msk[0:qr, :],
                                              negf[0:qr, :])
                    nc.vector.tensor_tensor(
                        out=ot[0:qr, 0:1], in0=ot[0:qr, 0:1],
                        in1=pad0_t[0:qr, qt:qt + 1], op=mybir.AluOpType.add)
                    r0 = b * L + qt * 128
                    nc.sync.dma_start(out[r0:r0 + qr, :], ot[0:qr, :])

    nc.compile()
    return nc


# ----------------------------------------------------------------------
# entry point
# ----------------------------------------------------------------------

def kernel(**inputs):
    import time as _time
    meta, in_maps = _host_prep(inputs)
    key = tuple(sorted(meta.items()))
    if key not in _CACHE:
        _CACHE[key] = _build(meta)
    nc = _CACHE[key]
    last = None
    for attempt in range(4):
        try:
            res = bass_utils.run_bass_kernel_spmd(
                nc, in_maps, core_ids=list(range(NCORES)))
            break
        except Exception as e:  # transient device wedge -> retry
            last = e
            if attempt == 3:
                raise
            _time.sleep(10 * (attempt + 1))
    return np.concatenate([res.results[c]["out"] for c in range(NCORES)],
                          axis=1)
